# revision 2
# baseline (speedup 1.0000x reference)
"""YOLOv1 loss kernel v2 for Trainium2, 8-core data-parallel, bf16.

Layout per core (1024 rows, chunks of g*128 rows, tiles [128, g, ch, 49]):
  pred  PP (30 ch): [x1,y1,x2,y2, w1,h1,w2,h2, c1,c2, cls*20]
  label LL (29 ch): [gx,gy, x5,y5, gw,gh, w7,h7, obj, cls*20]

Math: iou via 7x-scaled overlap ov = relu(min(7*min(w,wg), 3.5*(w+wg)-|dc|)),
ints = ovx*ovy, u49 = 49*(areas)-ints, iou = ints/u49.
All loss terms become sum_cells w_ch * d_ch^2 with per-cell weights:
  coor (D 0:8 = [c1x,c1y,c2x,c2y, s1w,s1h,s2w,s2h]): w = 5*obj*u / 5*obj*(1-u)
  conf (D 8:10 = dconf): w = obj*(.5+.5u) / obj*(1-.5u)
  cls  (D 10:30): w = obj
  noobj (pred conf raw, squared into SQ 30:32): w = .5*(1-obj)
Squares SQ = D^2 (Act; DVE self-mult on the last chunk); the weighted
accumulation runs on the idle PE: per 2-channel block,
matmul(psum[0:98,0:98], lhsT=weight-broadcast, rhs=SQ-block) accumulated over
all blocks/chunks; diag(psum)[i] = sum_p w[p,i]*sq[p,i]. Host sums the psum
diagonal (f64) -> loss.

Engines: DVE = diffs/iou pipeline; Pool = obj/weight builds/cls share;
Act = sqrt(5*w), squares; PE = weighted accumulation; SP = DMA.
cls diffs/squares pipeline per g-unit so PE streams behind Act.
"""

import sys

import numpy as np

for _p in ("/opt/trn_rl_repo", "/root/.axon_site/_ro/trn_rl_repo"):
    if _p not in sys.path:
        sys.path.insert(0, _p)

import concourse.bass as bass
import concourse.mybir as mybir

F32 = mybir.dt.float32
BF16 = mybir.dt.bfloat16
U16 = mybir.dt.uint16
Alu = mybir.AluOpType
Act = mybir.ActivationFunctionType

B_TOTAL = 8192
NCORES = 8
B_CORE = B_TOTAL // NCORES  # 1024
P = 128
K = 49
CP = 30
CL = 29
ROWP = CP * K
ROWL = CL * K

PP_IDX = [0, 1, 5, 6, 2, 3, 7, 8, 4, 9] + list(range(10, 30))
LL_IDX = [0, 1, 5, 6, 2, 3, 7, 8, 4] + list(range(10, 30))

PH = 10
LH = 9
CX = CP + CL          # 59 combined channels per row
HX = PH + LH          # 19 head channels
ROWX = CX * K

CFG = dict(
    chunks=(2, 3, 3),
    cls_pool=10,      # cls diff channels done on Pool (rest on DVE)
    sq_coor="act",    # coor squares on act or dve
    act_split=False,  # conf square separate from coor square on Act
)


def build_nc(cfg=None):
    cfg = dict(CFG, **(cfg or {}))
    chunks = cfg["chunks"]
    clsp = cfg["cls_pool"]
    clsd = 20 - clsp
    assert sum(chunks) * P == B_CORE
    nchunk = len(chunks)
    ilast = nchunk - 1
    maxg = max(chunks)
    nc = bass.Bass()
    x = nc.declare_dram_parameter("x", [B_CORE, ROWX], BF16, isOutput=False)
    out2 = nc.declare_dram_parameter("out2", [P, 98], F32, isOutput=True)

    from contextlib import ExitStack

    _ctr = [0]
    es = ExitStack()

    def sb(shape, dt=BF16):
        _ctr[0] += 1
        return es.enter_context(nc.sbuf_tensor(f"t{_ctr[0]}", shape, dt))

    with es:
        # combined tiles: xh = head (pred 0:10 | label 0:9),
        # xtl = tail (pred cls 20 | label cls 20)
        xh = [sb([P, maxg, HX, K]) for _ in range(2)]
        xtl = [sb([P, maxg, 40, K]) for _ in range(2)]
        XT = [sb([P, maxg, 4, K]) for _ in range(2)]
        AD = [sb([P, maxg, 4, K]) for _ in range(2)]
        SM = [sb([P, maxg, 8, K]) for _ in range(2)]
        SM2 = [sb([P, maxg, 8, K]) for _ in range(2)]
        TA = [sb([P, maxg, 4, K]) for _ in range(2)]
        OV = [sb([P, maxg, 4, K]) for _ in range(2)]
        CLt = [sb([P, maxg, 4, K]) for _ in range(2)]
        AR = [sb([P, maxg, 3, K]) for _ in range(2)]
        SA = [sb([P, maxg, 2, K]) for _ in range(2)]
        SA49 = [sb([P, maxg, 2, K]) for _ in range(2)]
        INT = [sb([P, maxg, 2, K]) for _ in range(2)]
        U49 = [sb([P, maxg, 2, K]) for _ in range(2)]
        RCPt = [sb([P, maxg, 2, K]) for _ in range(2)]
        IOU = [sb([P, maxg, 2, K]) for _ in range(2)]
        Ut = [sb([P, maxg, 1, K]) for _ in range(2)]
        O5 = [sb([P, maxg, 2, K]) for _ in range(2)]
        TQ = [sb([P, maxg, 2, K]) for _ in range(2)]
        # weight tile: [rm0,rm0 | rm1,rm1 | rm2,rm3 | rm4,rm4 | obj,obj]
        WT = [sb([P, maxg, 10, K]) for _ in range(2)]
        SQP = [sb([P, maxg, 4, K]) for _ in range(2)]
        SQL = [sb([P, maxg, 4, K]) for _ in range(2)]
        D = [sb([P, maxg, 30, K]) for _ in range(2)]
        SQ = [sb([P, maxg, 32, K]) for _ in range(2)]
        PSB = es.enter_context(nc.sbuf_tensor("psb", [P, 98], F32))
        ps = es.enter_context(nc.psum_tensor("ps", [P, 98], F32))

        sem_names = ["dsemHA", "dsemHB", "dsemTA", "dsemTB", "u_done",
                     "sqrt_done", "ds_done", "clsq", "rm_done", "sqA",
                     "pe_done", "dve_in", "act_in", "pool_in",
                     "cls_dve_g", "cls_pool_g", "obj_done", "rm4_done",
                     "sa_done", "dsemTL", "psb_done", "outd"]
        sems = {n: es.enter_context(nc.semaphore(n)) for n in sem_names}
        dsemH = [sems["dsemHA"], sems["dsemHB"]]
        dsemT = [sems["dsemTA"], sems["dsemTB"]]
        u_done = sems["u_done"]
        sqrt_done = sems["sqrt_done"]
        ds_done = sems["ds_done"]
        clsq = sems["clsq"]          # cls squares done, +1 per (chunk, gi)
        rm_done = sems["rm_done"]
        sqA = sems["sqA"]            # coor+conf+noobj squares, +1 per chunk
        pe_done = sems["pe_done"]
        dve_in = sems["dve_in"]
        act_in = sems["act_in"]
        pool_in = sems["pool_in"]
        cls_dve_g = sems["cls_dve_g"]    # cls diffs (dve share) per (chunk,gi)
        cls_pool_g = sems["cls_pool_g"]  # cls diffs (pool share) per (chunk,gi)
        obj_done = sems["obj_done"]
        rm4_done = sems["rm4_done"]
        sa_done = sems["sa_done"]
        dsemTL = sems["dsemTL"]
        psb_done = sems["psb_done"]
        outd = sems["outd"]

        block = es.enter_context(nc.Block())

        offs = [0]
        for g in chunks:
            offs.append(offs[-1] + g * P)

        headv = {}
        tailv = {}
        _hv = [0, 0]
        _tv = [0, 0]
        for i in range(nchunk):
            s = i % 2
            _hv[s] += 32
            _tv[s] += 32
            headv[i] = _hv[s]
            tailv[i] = _tv[s]
        # cumulative (chunk, gi) counter base: number of g-units before chunk i
        gbase = [sum(chunks[:i]) for i in range(nchunk)]

        @block.sync
        def _(sync):
            def head(i):
                g = chunks[i]
                s = i % 2
                if i >= 2:
                    # xh[s] readers of chunk i-2: dve dconf (<= u_done),
                    # act sqno (act_in), pool obj (obj_done)
                    sync.wait_ge(u_done, i - 1)
                    sync.wait_ge(act_in, i - 1)
                    sync.wait_ge(obj_done, i - 1)
                rows = slice(offs[i], offs[i + 1])
                sync.dma_start(
                    out=xh[s][:, 0:g].rearrange("p g c k -> p g (c k)"),
                    in_=x[rows, 0:HX * K].rearrange("(g p) d -> p g d", p=P),
                ).then_inc(dsemH[s], 32)

            def tail(i):
                g = chunks[i]
                s = i % 2
                if i >= 2:
                    # xtl[s] readers of chunk i-2: dve + pool cls diffs
                    sync.wait_ge(cls_dve_g, gbase[i - 1])
                    if clsp > 0:
                        sync.wait_ge(cls_pool_g, gbase[i - 1])
                rows = slice(offs[i], offs[i + 1])
                if i == ilast:
                    for gi in range(g):
                        rg = slice(offs[i] + gi * P, offs[i] + (gi + 1) * P)
                        sync.dma_start(
                            out=xtl[s][:, gi].rearrange("p c k -> p (c k)"),
                            in_=x[rg, HX * K:].rearrange(
                                "(g p) d -> p (g d)", p=P),
                        ).then_inc(dsemTL, 16)
                else:
                    sync.dma_start(
                        out=xtl[s][:, 0:g].rearrange("p g c k -> p g (c k)"),
                        in_=x[rows, HX * K:].rearrange("(g p) d -> p g d", p=P),
                    ).then_inc(dsemT[s], 32)

            for i in range(nchunk):
                head(i)
                tail(i)
            sync.wait_ge(psb_done, 1)
            sync.dma_start(out=out2[:], in_=PSB[:]).then_inc(outd, 16)
            sync.wait_ge(outd, 16)

        @block.gpsimd
        def _(gp):
            for i, g in enumerate(chunks):
                s = i % 2
                if i >= 2:
                    gp.wait_ge(pe_done, i - 1)
                gp.wait_ge(dsemH[s], headv[i])
                gp.tensor_scalar(WT[s][:, 0:g, 8:10, :],
                                 xh[s][:, 0:g, 18:19, :].broadcast_to(
                                     [P, g, 2, K]),
                                 1.0, None,
                                 Alu.is_equal).then_inc(obj_done, 1)
                gp.drain()
                gp.tensor_scalar(WT[s][:, 0:g, 6:8, :], WT[s][:, 0:g, 8:10, :],
                                 -0.5, 0.5, Alu.mult,
                                 Alu.add).then_inc(rm4_done, 1)
                gp.drain().then_inc(pool_in, 1)

                def pool_cls(j):
                    sj = j % 2
                    if clsp == 0:
                        return
                    if j >= 2:
                        gp.wait_ge(sqA, j - 1)
                        gp.wait_ge(clsq, gbase[j - 1])
                    for gi in range(chunks[j]):
                        if j == ilast:
                            gp.wait_ge(dsemTL, 16 * (gi + 1))
                        elif gi == 0:
                            gp.wait_ge(dsemT[sj], tailv[j])
                        gp.tensor_tensor(
                            D[sj][:, gi, 10 + clsd:30, :],
                            xtl[sj][:, gi, clsd:20, :],
                            xtl[sj][:, gi, 20 + clsd:40, :], Alu.subtract,
                        ).then_inc(cls_pool_g, 1)
                    gp.drain()

                pool_cls(i)
                if i == ilast:
                    continue  # last-chunk rm built on DVE
                gp.wait_ge(u_done, i + 1)
                gp.tensor_scalar(O5[s][:, 0:g], WT[s][:, 0:g, 8:10, :],
                                 5.0, None, Alu.mult)
                gp.drain()
                gp.tensor_tensor(WT[s][:, 0:g, 0:2, :], O5[s][:, 0:g],
                                 Ut[s][:, 0:g].broadcast_to([P, g, 2, K]),
                                 Alu.mult)
                gp.tensor_scalar(TQ[s][:, 0:g, 0:1, :], Ut[s][:, 0:g],
                                 0.5, 0.5, Alu.mult, Alu.add)
                gp.tensor_scalar(TQ[s][:, 0:g, 1:2, :], Ut[s][:, 0:g],
                                 -0.5, 1.0, Alu.mult, Alu.add)
                gp.drain()
                gp.tensor_tensor(WT[s][:, 0:g, 2:4, :], O5[s][:, 0:g],
                                 WT[s][:, 0:g, 0:2, :], Alu.subtract)
                gp.tensor_tensor(
                    WT[s][:, 0:g, 4:6, :], TQ[s][:, 0:g],
                    WT[s][:, 0:g, 8:10, :], Alu.mult,
                ).then_inc(rm_done, 1)
                gp.drain()

        @block.scalar
        def _(act):
            for i, g in enumerate(chunks):
                s = i % 2
                if i >= 2:
                    act.wait_ge(pe_done, i - 1)   # SQ reuse
                    act.wait_ge(ds_done, i - 1)   # SQP/SQL reuse
                act.wait_ge(dsemH[s], headv[i])
                act.activation(SQP[s][:, 0:g], xh[s][:, 0:g, 4:8, :],
                               Act.Sqrt)
                act.activation(SQL[s][:, 0:g], xh[s][:, 0:g, 14:18, :],
                               Act.Sqrt).then_inc(sqrt_done, 1)
                act.activation(SQ[s][:, 0:g, 30:32, :],
                               xh[s][:, 0:g, 8:10, :],
                               Act.Square).then_inc(act_in, 1)
                for gi in range(g):
                    act.wait_ge(cls_dve_g, gbase[i] + gi + 1)
                    if clsp > 0:
                        act.wait_ge(cls_pool_g, gbase[i] + gi + 1)
                    act.activation(SQ[s][:, gi, 10:30, :],
                                   D[s][:, gi, 10:30, :],
                                   Act.Square).then_inc(clsq, 1)
                act.wait_ge(u_done, i + 1)
                act.wait_ge(ds_done, i + 1)
                act.activation(SQ[s][:, 0:g, 0:10, :],
                               D[s][:, 0:g, 0:10, :],
                               Act.Square).then_inc(sqA, 1)
            act.wait_ge(pe_done, nchunk)
            act.activation(PSB[0:98, :], ps[0:98, :],
                           Act.Copy).then_inc(psb_done, 1)

        @block.vector
        def _(v):
            tt = v.tensor_tensor
            ts = v.tensor_scalar

            def cls_fill(j, gi):
                """emit chunk j's cls diff for unit gi (waits its tail dma)"""
                sj = j % 2
                if j == ilast:
                    v.wait_ge(dsemTL, 16 * (gi + 1))
                elif gi == 0:
                    v.wait_ge(dsemT[sj], tailv[j])
                tt(D[sj][:, gi, 10:10 + clsd, :],
                   xtl[sj][:, gi, 0:clsd, :],
                   xtl[sj][:, gi, 20:20 + clsd, :],
                   Alu.subtract).then_inc(cls_dve_g, 1)

            for i, g in enumerate(chunks):
                s = i % 2
                last = (i == ilast)
                p = xh[s]
                # cls fills ride in this chunk's own chain drain slots
                fills = [] if last else [(i, gi) for gi in range(g)]
                if i >= 2:
                    v.wait_ge(sqA, i - 1)      # D reuse (act read done)
                    v.wait_ge(clsq, gbase[i - 1])  # prev-parity cls read
                    v.wait_ge(rm_done, i - 1)  # Ut reuse
                    if cfg["sq_coor"] == "dve":
                        v.wait_ge(pe_done, i - 1)  # SQ reuse
                if last:
                    # cls diffs first: tail DMA streams while the previous
                    # chunk still computes; Act/PE stream cls early
                    for gi in range(g):
                        cls_fill(i, gi)
                v.wait_ge(dsemH[s], headv[i])
                gxy2 = (p[:, 0:g, 10:12, :].unsqueeze(2)
                        .broadcast_to([P, g, 2, 2, K]))
                gwh2 = (p[:, 0:g, 14:16, :].unsqueeze(2)
                        .broadcast_to([P, g, 2, 2, K]))
                pxy_v = p[:, 0:g, 0:4, :].rearrange(
                    "p g (b c) k -> p g b c k", b=2)
                pwh_v = p[:, 0:g, 4:8, :].rearrange(
                    "p g (b c) k -> p g b c k", b=2)
                # W1
                tt(XT[s][:, 0:g].rearrange("p g (b c) k -> p g b c k", b=2),
                   pxy_v, gxy2, Alu.subtract)
                tt(D[s][:, 0:g, 2:4, :], p[:, 0:g, 2:4, :],
                   p[:, 0:g, 12:14, :], Alu.subtract)
                tt(SM[s][:, 0:g, 0:4, :].rearrange(
                    "p g (b c) k -> p g b c k", b=2),
                   pwh_v, gwh2, Alu.add)
                tt(SM[s][:, 0:g, 4:8, :].rearrange(
                    "p g (b c) k -> p g b c k", b=2),
                   pwh_v, gwh2, Alu.min)
                tt(AR[s][:, 0:g, 0:2, :],
                   pwh_v[:, :, :, 0, :], pwh_v[:, :, :, 1, :], Alu.mult)
                tt(AR[s][:, 0:g, 2:3, :], p[:, 0:g, 14:15, :],
                   p[:, 0:g, 15:16, :], Alu.mult)
                v.drain()
                # W1b
                v.tensor_copy(D[s][:, 0:g, 0:2, :], XT[s][:, 0:g, 0:2, :])
                ts(AD[s][:, 0:g].bitcast(U16), XT[s][:, 0:g].bitcast(U16),
                   0x7FFF, None, Alu.bitwise_and)
                ts(SM2[s][:, 0:g, 0:4, :], SM[s][:, 0:g, 0:4, :],
                   3.5, None, Alu.mult)
                ts(SM2[s][:, 0:g, 4:8, :], SM[s][:, 0:g, 4:8, :],
                   7.0, None, Alu.mult)
                tt(SA[s][:, 0:g], AR[s][:, 0:g, 0:2, :],
                   AR[s][:, 0:g, 2:3, :].broadcast_to([P, g, 2, K]),
                   Alu.add)
                v.drain()
                # W2 chain; prev chunk's cls diffs + ds fill the drain slots
                tt(TA[s][:, 0:g], SM2[s][:, 0:g, 0:4, :], AD[s][:, 0:g],
                   Alu.subtract)
                v.wait_ge(sqrt_done, i + 1)
                tt(D[s][:, 0:g, 4:8, :], SQP[s][:, 0:g], SQL[s][:, 0:g],
                   Alu.subtract).then_inc(ds_done, 1)
                v.drain()
                tt(OV[s][:, 0:g], SM2[s][:, 0:g, 4:8, :], TA[s][:, 0:g],
                   Alu.min)
                if fills:
                    cls_fill(*fills.pop(0))
                v.drain()
                ts(CLt[s][:, 0:g], OV[s][:, 0:g], 0.0, None, Alu.max)
                if fills:
                    cls_fill(*fills.pop(0))
                v.drain()
                clv = CLt[s][:, 0:g].rearrange("p g (b c) k -> p g b c k",
                                               b=2)
                tt(INT[s][:, 0:g], clv[:, :, :, 0, :], clv[:, :, :, 1, :],
                   Alu.mult)
                if fills:
                    cls_fill(*fills.pop(0))
                v.drain()
                ts(SA49[s][:, 0:g], SA[s][:, 0:g], 49.0, None, Alu.mult)
                if fills:
                    cls_fill(*fills.pop(0))
                v.drain()
                tt(U49[s][:, 0:g], SA49[s][:, 0:g], INT[s][:, 0:g],
                   Alu.subtract)
                v.drain()
                with nc.allow_low_precision(reason="bf16 iou ok"):
                    v.reciprocal(RCPt[s][:, 0:g], U49[s][:, 0:g])
                v.drain()
                tt(IOU[s][:, 0:g], INT[s][:, 0:g], RCPt[s][:, 0:g], Alu.mult)
                v.drain()
                tt(Ut[s][:, 0:g], IOU[s][:, 0:g, 0:1, :],
                   IOU[s][:, 0:g, 1:2, :], Alu.is_ge)
                tt(D[s][:, 0:g, 8:10, :], p[:, 0:g, 8:10, :], IOU[s][:, 0:g],
                   Alu.subtract)
                v.drain().then_inc(u_done, 1)
                if cfg["sq_coor"] == "dve":
                    tt(SQ[s][:, 0:g, 0:8, :], D[s][:, 0:g, 0:8, :],
                       D[s][:, 0:g, 0:8, :], Alu.mult).then_inc(sqA, 1)
                if not last:
                    while fills:
                        cls_fill(*fills.pop(0))
                    v.drain().then_inc(dve_in, 1)
                else:
                    # last chunk: rm build on DVE (shortest tail)
                    v.wait_ge(obj_done, i + 1)
                    ts(TQ[s][:, 0:g, 0:1, :], Ut[s][:, 0:g],
                       0.5, 0.5, Alu.mult, Alu.add)
                    ts(TQ[s][:, 0:g, 1:2, :], Ut[s][:, 0:g],
                       -0.5, 1.0, Alu.mult, Alu.add)
                    ts(O5[s][:, 0:g], WT[s][:, 0:g, 8:10, :],
                       5.0, None, Alu.mult)
                    v.drain().then_inc(dve_in, 1)
                    tt(WT[s][:, 0:g, 0:2, :], O5[s][:, 0:g],
                       Ut[s][:, 0:g].broadcast_to([P, g, 2, K]), Alu.mult)
                    tt(WT[s][:, 0:g, 4:6, :], TQ[s][:, 0:g],
                       WT[s][:, 0:g, 8:10, :], Alu.mult)
                    v.drain()
                    tt(WT[s][:, 0:g, 2:4, :], O5[s][:, 0:g],
                       WT[s][:, 0:g, 0:2, :],
                       Alu.subtract).then_inc(rm_done, 1)
                    v.drain()
                    while fills:
                        cls_fill(*fills.pop(0))
                    v.drain()

        @block.tensor
        def _(pe):
            first = True
            for i, g in enumerate(chunks):
                s = i % 2

                def mm(c, w, gi, last=False):
                    nonlocal first
                    r = pe.matmul(
                        ps[0:98, :],
                        WT[s][:, gi, w:w + 2, :].rearrange(
                            "p c k -> p (c k)"),
                        SQ[s][:, gi, c:c + 2, :].rearrange(
                            "p c k -> p (c k)"),
                        start=first, stop=last, skip_group_check=True)
                    first = False
                    return r

                # wave B: cls blocks per gi (squares stream in per g-unit)
                for gi in range(g):
                    pe.wait_ge(clsq, gbase[i] + gi + 1)
                    if gi == 0:
                        pe.wait_ge(rm4_done, i + 1)
                    for c in range(10, 30, 2):
                        mm(c, 8, gi)
                # wave A: coor + conf + noobj per chunk
                pe.wait_ge(sqA, i + 1)
                pe.wait_ge(rm_done, i + 1)
                for gi in range(g):
                    mm(0, 0, gi)
                    mm(2, 2, gi)
                    mm(4, 0, gi)
                    mm(6, 2, gi)
                    mm(8, 4, gi)
                    lastmm = (i == nchunk - 1 and gi == g - 1)
                    r = mm(30, 6, gi, last=lastmm)
                    if gi == g - 1:
                        r.then_inc(pe_done, 1)

    return nc


_NC_CACHE = {}


def _get_nc():
    if "nc" not in _NC_CACHE:
        _NC_CACHE["nc"] = build_nc()
    return _NC_CACHE["nc"]


def _to_bf16_repack(pred, labels):
    import ml_dtypes

    bf = ml_dtypes.bfloat16
    p = np.ascontiguousarray(pred, dtype=np.float32).reshape(B_TOTAL, 30, K)
    l = np.ascontiguousarray(labels, dtype=np.float32).reshape(B_TOTAL, 30, K)
    pb = p.astype(bf)
    lb = l.astype(bf)
    # obj channel: keep the ==1.0 test exact under rounding
    l4 = l[:, 4, :]
    lb4 = lb[:, 4, :]
    bad = (l4 != np.float32(1.0)) & (lb4.astype(np.float32) == np.float32(1.0))
    if bad.any():
        lb4[bad] = bf(0.99609375)
        lb[:, 4, :] = lb4
    xall = np.concatenate(
        [pb[:, PP_IDX[:PH], :], lb[:, LL_IDX[:LH], :],
         pb[:, PP_IDX[PH:], :], lb[:, LL_IDX[LH:], :]], axis=1)
    return np.ascontiguousarray(xall).reshape(B_TOTAL, ROWX)


def run_device(pred, labels, trace=False):
    from concourse.bass_utils import run_bass_kernel_spmd

    nc = _get_nc()
    xrp = _to_bf16_repack(pred, labels)
    in_maps = []
    for c in range(NCORES):
        rows = slice(c * B_CORE, (c + 1) * B_CORE)
        in_maps.append({"x": xrp[rows]})
    res = run_bass_kernel_spmd(nc, in_maps, list(range(NCORES)), trace=trace)
    total = 0.0
    for c in range(NCORES):
        m = res.results[c]["out2"].astype(np.float64)
        total += float(np.trace(m[0:98, 0:98]))
    loss = np.float32(total / B_TOTAL)
    return loss, res


def kernel(pred, labels):
    loss, _ = run_device(pred, labels, trace=False)
    return np.array(loss, dtype=np.float32)


if __name__ == "__main__":
    rng = np.random.default_rng(0)
    p = rng.random((B_TOTAL, 30, 7, 7), dtype=np.float32)
    l = rng.random((B_TOTAL, 30, 7, 7), dtype=np.float32)
    l[:, 4] = (rng.random((B_TOTAL, 7, 7)) < 0.3).astype(np.float32)
    print(kernel(p, l))


# revision 3
# speedup vs baseline: 1.0142x; 1.0142x over previous
"""YOLOv1 loss kernel v2 for Trainium2, 8-core data-parallel, bf16.

Layout per core (1024 rows, chunks of g*128 rows, tiles [128, g, ch, 49]):
  pred  PP (30 ch): [x1,y1,x2,y2, w1,h1,w2,h2, c1,c2, cls*20]
  label LL (29 ch): [gx,gy, x5,y5, gw,gh, w7,h7, obj, cls*20]

Math: iou via 7x-scaled overlap ov = relu(min(7*min(w,wg), 3.5*(w+wg)-|dc|)),
ints = ovx*ovy, u49 = 49*(areas)-ints, iou = ints/u49.
All loss terms become sum_cells w_ch * d_ch^2 with per-cell weights:
  coor (D 0:8 = [c1x,c1y,c2x,c2y, s1w,s1h,s2w,s2h]): w = 5*obj*u / 5*obj*(1-u)
  conf (D 8:10 = dconf): w = obj*(.5+.5u) / obj*(1-.5u)
  cls  (D 10:30): w = obj
  noobj (pred conf raw, squared into SQ 30:32): w = .5*(1-obj)
Squares SQ = D^2 (Act; DVE self-mult on the last chunk); the weighted
accumulation runs on the idle PE: per 2-channel block,
matmul(psum[0:98,0:98], lhsT=weight-broadcast, rhs=SQ-block) accumulated over
all blocks/chunks; diag(psum)[i] = sum_p w[p,i]*sq[p,i]. Host sums the psum
diagonal (f64) -> loss.

Engines: DVE = diffs/iou pipeline; Pool = obj/weight builds/cls share;
Act = sqrt(5*w), squares; PE = weighted accumulation; SP = DMA.
cls diffs/squares pipeline per g-unit so PE streams behind Act.
"""

import sys

import numpy as np

for _p in ("/opt/trn_rl_repo", "/root/.axon_site/_ro/trn_rl_repo"):
    if _p not in sys.path:
        sys.path.insert(0, _p)

import concourse.bass as bass
import concourse.mybir as mybir

F32 = mybir.dt.float32
BF16 = mybir.dt.bfloat16
U16 = mybir.dt.uint16
Alu = mybir.AluOpType
Act = mybir.ActivationFunctionType

B_TOTAL = 8192
NCORES = 8
B_CORE = B_TOTAL // NCORES  # 1024
P = 128
K = 49
CP = 30
CL = 29
ROWP = CP * K
ROWL = CL * K

PP_IDX = [0, 1, 5, 6, 2, 3, 7, 8, 4, 9] + list(range(10, 30))
LL_IDX = [0, 1, 5, 6, 2, 3, 7, 8, 4] + list(range(10, 30))

PH = 10
LH = 9
CX = CP + CL          # 59 combined channels per row
HX = PH + LH          # 19 head channels
ROWX = CX * K

CFG = dict(
    chunks=(2, 3, 3),
    cls_pool=9,       # cls diff channels done on Pool (rest on DVE)
    sq_coor="act",    # coor squares on act or dve
    act_split=False,  # conf square separate from coor square on Act
)


def build_nc(cfg=None):
    cfg = dict(CFG, **(cfg or {}))
    chunks = cfg["chunks"]
    clsp = cfg["cls_pool"]
    clsd = 20 - clsp
    assert sum(chunks) * P == B_CORE
    nchunk = len(chunks)
    ilast = nchunk - 1
    maxg = max(chunks)
    nc = bass.Bass()
    x = nc.declare_dram_parameter("x", [B_CORE, ROWX], BF16, isOutput=False)
    out2 = nc.declare_dram_parameter("out2", [P, 98], F32, isOutput=True)

    from contextlib import ExitStack

    _ctr = [0]
    es = ExitStack()

    def sb(shape, dt=BF16):
        _ctr[0] += 1
        return es.enter_context(nc.sbuf_tensor(f"t{_ctr[0]}", shape, dt))

    with es:
        # combined tiles: xh = head (pred 0:10 | label 0:9),
        # xtl = tail (pred cls 20 | label cls 20)
        xh = [sb([P, maxg, HX, K]) for _ in range(2)]
        xtl = [sb([P, maxg, 40, K]) for _ in range(2)]
        XT = [sb([P, maxg, 4, K]) for _ in range(2)]
        AD = [sb([P, maxg, 4, K]) for _ in range(2)]
        SM = [sb([P, maxg, 8, K]) for _ in range(2)]
        SM2 = [sb([P, maxg, 8, K]) for _ in range(2)]
        TA = [sb([P, maxg, 4, K]) for _ in range(2)]
        OV = [sb([P, maxg, 4, K]) for _ in range(2)]
        CLt = [sb([P, maxg, 4, K]) for _ in range(2)]
        AR = [sb([P, maxg, 3, K]) for _ in range(2)]
        SA = [sb([P, maxg, 2, K]) for _ in range(2)]
        SA49 = [sb([P, maxg, 2, K]) for _ in range(2)]
        INT = [sb([P, maxg, 2, K]) for _ in range(2)]
        U49 = [sb([P, maxg, 2, K]) for _ in range(2)]
        RCPt = [sb([P, maxg, 2, K]) for _ in range(2)]
        IOU = [sb([P, maxg, 2, K]) for _ in range(2)]
        Ut = [sb([P, maxg, 1, K]) for _ in range(2)]
        O5 = [sb([P, maxg, 2, K]) for _ in range(2)]
        TQ = [sb([P, maxg, 2, K]) for _ in range(2)]
        # weight tile: [rm0,rm0 | rm1,rm1 | rm2,rm3 | rm4,rm4 | obj,obj]
        WT = [sb([P, maxg, 10, K]) for _ in range(2)]
        SQP = [sb([P, maxg, 4, K]) for _ in range(2)]
        SQL = [sb([P, maxg, 4, K]) for _ in range(2)]
        D = [sb([P, maxg, 30, K]) for _ in range(2)]
        SQ = [sb([P, maxg, 32, K]) for _ in range(2)]
        PSB = es.enter_context(nc.sbuf_tensor("psb", [P, 98], F32))
        ps = es.enter_context(nc.psum_tensor("ps", [P, 98], F32))

        sem_names = ["dsemHA", "dsemHB", "dsemTA", "dsemTB", "u_done",
                     "sqrt_done", "ds_done", "clsq", "rm_done", "sqA",
                     "pe_done", "dve_in", "act_in", "pool_in",
                     "cls_dve_g", "cls_pool_g", "obj_done", "rm4_done",
                     "sa_done", "dsemTL", "psb_done", "outd"]
        sems = {n: es.enter_context(nc.semaphore(n)) for n in sem_names}
        dsemH = [sems["dsemHA"], sems["dsemHB"]]
        dsemT = [sems["dsemTA"], sems["dsemTB"]]
        u_done = sems["u_done"]
        sqrt_done = sems["sqrt_done"]
        ds_done = sems["ds_done"]
        clsq = sems["clsq"]          # cls squares done, +1 per (chunk, gi)
        rm_done = sems["rm_done"]
        sqA = sems["sqA"]            # coor+conf+noobj squares, +1 per chunk
        pe_done = sems["pe_done"]
        dve_in = sems["dve_in"]
        act_in = sems["act_in"]
        pool_in = sems["pool_in"]
        cls_dve_g = sems["cls_dve_g"]    # cls diffs (dve share) per (chunk,gi)
        cls_pool_g = sems["cls_pool_g"]  # cls diffs (pool share) per (chunk,gi)
        obj_done = sems["obj_done"]
        rm4_done = sems["rm4_done"]
        sa_done = sems["sa_done"]
        dsemTL = sems["dsemTL"]
        psb_done = sems["psb_done"]
        outd = sems["outd"]

        block = es.enter_context(nc.Block())

        offs = [0]
        for g in chunks:
            offs.append(offs[-1] + g * P)

        headv = {}
        tailv = {}
        _hv = [0, 0]
        _tv = [0, 0]
        for i in range(nchunk):
            s = i % 2
            _hv[s] += 32
            _tv[s] += 32
            headv[i] = _hv[s]
            tailv[i] = _tv[s]
        # cumulative (chunk, gi) counter base: number of g-units before chunk i
        gbase = [sum(chunks[:i]) for i in range(nchunk)]

        @block.sync
        def _(sync):
            def head(i):
                g = chunks[i]
                s = i % 2
                if i >= 2:
                    # xh[s] readers of chunk i-2: dve dconf (<= u_done),
                    # act sqno (act_in), pool obj (obj_done)
                    sync.wait_ge(u_done, i - 1)
                    sync.wait_ge(act_in, i - 1)
                    sync.wait_ge(obj_done, i - 1)
                rows = slice(offs[i], offs[i + 1])
                sync.dma_start(
                    out=xh[s][:, 0:g].rearrange("p g c k -> p g (c k)"),
                    in_=x[rows, 0:HX * K].rearrange("(g p) d -> p g d", p=P),
                ).then_inc(dsemH[s], 32)

            def tail(i):
                g = chunks[i]
                s = i % 2
                if i >= 2:
                    # xtl[s] readers of chunk i-2: dve + pool cls diffs
                    sync.wait_ge(cls_dve_g, gbase[i - 1])
                    if clsp > 0:
                        sync.wait_ge(cls_pool_g, gbase[i - 1])
                rows = slice(offs[i], offs[i + 1])
                if i == ilast:
                    for gi in range(g):
                        rg = slice(offs[i] + gi * P, offs[i] + (gi + 1) * P)
                        sync.dma_start(
                            out=xtl[s][:, gi].rearrange("p c k -> p (c k)"),
                            in_=x[rg, HX * K:].rearrange(
                                "(g p) d -> p (g d)", p=P),
                        ).then_inc(dsemTL, 16)
                else:
                    sync.dma_start(
                        out=xtl[s][:, 0:g].rearrange("p g c k -> p g (c k)"),
                        in_=x[rows, HX * K:].rearrange("(g p) d -> p g d", p=P),
                    ).then_inc(dsemT[s], 32)

            for i in range(nchunk):
                head(i)
                tail(i)
            sync.wait_ge(outd, 16)

        @block.gpsimd
        def _(gp):
            for i, g in enumerate(chunks):
                s = i % 2
                if i >= 2:
                    gp.wait_ge(pe_done, i - 1)
                gp.wait_ge(dsemH[s], headv[i])
                gp.tensor_scalar(WT[s][:, 0:g, 8:10, :],
                                 xh[s][:, 0:g, 18:19, :].broadcast_to(
                                     [P, g, 2, K]),
                                 1.0, None,
                                 Alu.is_equal).then_inc(obj_done, 1)
                gp.drain()
                gp.tensor_scalar(WT[s][:, 0:g, 6:8, :], WT[s][:, 0:g, 8:10, :],
                                 -0.5, 0.5, Alu.mult,
                                 Alu.add).then_inc(rm4_done, 1)
                gp.drain().then_inc(pool_in, 1)

                def pool_cls(j):
                    sj = j % 2
                    if clsp == 0:
                        return
                    if j >= 2:
                        gp.wait_ge(sqA, j - 1)
                        gp.wait_ge(clsq, gbase[j - 1])
                    for gi in range(chunks[j]):
                        if j == ilast:
                            gp.wait_ge(dsemTL, 16 * (gi + 1))
                        elif gi == 0:
                            gp.wait_ge(dsemT[sj], tailv[j])
                        gp.tensor_tensor(
                            D[sj][:, gi, 10 + clsd:30, :],
                            xtl[sj][:, gi, clsd:20, :],
                            xtl[sj][:, gi, 20 + clsd:40, :], Alu.subtract,
                        ).then_inc(cls_pool_g, 1)
                    gp.drain()

                pool_cls(i)
                if i == ilast:
                    continue  # last-chunk rm built on DVE
                gp.wait_ge(u_done, i + 1)
                gp.tensor_scalar(O5[s][:, 0:g], WT[s][:, 0:g, 8:10, :],
                                 5.0, None, Alu.mult)
                gp.drain()
                gp.tensor_tensor(WT[s][:, 0:g, 0:2, :], O5[s][:, 0:g],
                                 Ut[s][:, 0:g].broadcast_to([P, g, 2, K]),
                                 Alu.mult)
                gp.tensor_scalar(TQ[s][:, 0:g, 0:1, :], Ut[s][:, 0:g],
                                 0.5, 0.5, Alu.mult, Alu.add)
                gp.tensor_scalar(TQ[s][:, 0:g, 1:2, :], Ut[s][:, 0:g],
                                 -0.5, 1.0, Alu.mult, Alu.add)
                gp.drain()
                gp.tensor_tensor(WT[s][:, 0:g, 2:4, :], O5[s][:, 0:g],
                                 WT[s][:, 0:g, 0:2, :], Alu.subtract)
                gp.tensor_tensor(
                    WT[s][:, 0:g, 4:6, :], TQ[s][:, 0:g],
                    WT[s][:, 0:g, 8:10, :], Alu.mult,
                ).then_inc(rm_done, 1)
                gp.drain()

        @block.scalar
        def _(act):
            for i, g in enumerate(chunks):
                s = i % 2
                if i >= 2:
                    act.wait_ge(pe_done, i - 1)   # SQ reuse
                    act.wait_ge(ds_done, i - 1)   # SQP/SQL reuse
                act.wait_ge(dsemH[s], headv[i])
                act.activation(SQP[s][:, 0:g], xh[s][:, 0:g, 4:8, :],
                               Act.Sqrt)
                act.activation(SQL[s][:, 0:g], xh[s][:, 0:g, 14:18, :],
                               Act.Sqrt).then_inc(sqrt_done, 1)
                act.activation(SQ[s][:, 0:g, 30:32, :],
                               xh[s][:, 0:g, 8:10, :],
                               Act.Square).then_inc(act_in, 1)
                for gi in range(g):
                    act.wait_ge(cls_dve_g, gbase[i] + gi + 1)
                    if clsp > 0:
                        act.wait_ge(cls_pool_g, gbase[i] + gi + 1)
                    act.activation(SQ[s][:, gi, 10:30, :],
                                   D[s][:, gi, 10:30, :],
                                   Act.Square).then_inc(clsq, 1)
                act.wait_ge(u_done, i + 1)
                act.wait_ge(ds_done, i + 1)
                act.activation(SQ[s][:, 0:g, 0:10, :],
                               D[s][:, 0:g, 0:10, :],
                               Act.Square).then_inc(sqA, 1)
            act.wait_ge(pe_done, nchunk)
            act.activation(PSB[0:98, :], ps[0:98, :],
                           Act.Copy).then_inc(psb_done, 1)
            act.dma_start(out=out2[:], in_=PSB[:]).then_inc(outd, 16)

        @block.vector
        def _(v):
            tt = v.tensor_tensor
            ts = v.tensor_scalar

            def cls_fill(j, gi):
                """emit chunk j's cls diff for unit gi (waits its tail dma)"""
                sj = j % 2
                if j == ilast:
                    v.wait_ge(dsemTL, 16 * (gi + 1))
                elif gi == 0:
                    v.wait_ge(dsemT[sj], tailv[j])
                tt(D[sj][:, gi, 10:10 + clsd, :],
                   xtl[sj][:, gi, 0:clsd, :],
                   xtl[sj][:, gi, 20:20 + clsd, :],
                   Alu.subtract).then_inc(cls_dve_g, 1)

            for i, g in enumerate(chunks):
                s = i % 2
                last = (i == ilast)
                p = xh[s]
                # cls fills ride in this chunk's own chain drain slots
                fills = [] if last else [(i, gi) for gi in range(g)]
                if i >= 2:
                    v.wait_ge(sqA, i - 1)      # D reuse (act read done)
                    v.wait_ge(clsq, gbase[i - 1])  # prev-parity cls read
                    v.wait_ge(rm_done, i - 1)  # Ut reuse
                    if cfg["sq_coor"] == "dve":
                        v.wait_ge(pe_done, i - 1)  # SQ reuse
                if last:
                    # cls diffs first: tail DMA streams while the previous
                    # chunk still computes; Act/PE stream cls early
                    for gi in range(g):
                        cls_fill(i, gi)
                v.wait_ge(dsemH[s], headv[i])
                gxy2 = (p[:, 0:g, 10:12, :].unsqueeze(2)
                        .broadcast_to([P, g, 2, 2, K]))
                gwh2 = (p[:, 0:g, 14:16, :].unsqueeze(2)
                        .broadcast_to([P, g, 2, 2, K]))
                pxy_v = p[:, 0:g, 0:4, :].rearrange(
                    "p g (b c) k -> p g b c k", b=2)
                pwh_v = p[:, 0:g, 4:8, :].rearrange(
                    "p g (b c) k -> p g b c k", b=2)
                # W1
                tt(XT[s][:, 0:g].rearrange("p g (b c) k -> p g b c k", b=2),
                   pxy_v, gxy2, Alu.subtract)
                tt(D[s][:, 0:g, 2:4, :], p[:, 0:g, 2:4, :],
                   p[:, 0:g, 12:14, :], Alu.subtract)
                tt(SM[s][:, 0:g, 0:4, :].rearrange(
                    "p g (b c) k -> p g b c k", b=2),
                   pwh_v, gwh2, Alu.add)
                tt(SM[s][:, 0:g, 4:8, :].rearrange(
                    "p g (b c) k -> p g b c k", b=2),
                   pwh_v, gwh2, Alu.min)
                tt(AR[s][:, 0:g, 0:2, :],
                   pwh_v[:, :, :, 0, :], pwh_v[:, :, :, 1, :], Alu.mult)
                tt(AR[s][:, 0:g, 2:3, :], p[:, 0:g, 14:15, :],
                   p[:, 0:g, 15:16, :], Alu.mult)
                v.drain()
                # W1b
                v.tensor_copy(D[s][:, 0:g, 0:2, :], XT[s][:, 0:g, 0:2, :])
                ts(AD[s][:, 0:g].bitcast(U16), XT[s][:, 0:g].bitcast(U16),
                   0x7FFF, None, Alu.bitwise_and)
                ts(SM2[s][:, 0:g, 0:4, :], SM[s][:, 0:g, 0:4, :],
                   3.5, None, Alu.mult)
                ts(SM2[s][:, 0:g, 4:8, :], SM[s][:, 0:g, 4:8, :],
                   7.0, None, Alu.mult)
                tt(SA[s][:, 0:g], AR[s][:, 0:g, 0:2, :],
                   AR[s][:, 0:g, 2:3, :].broadcast_to([P, g, 2, K]),
                   Alu.add)
                v.drain()
                # W2 chain; prev chunk's cls diffs + ds fill the drain slots
                tt(TA[s][:, 0:g], SM2[s][:, 0:g, 0:4, :], AD[s][:, 0:g],
                   Alu.subtract)
                v.wait_ge(sqrt_done, i + 1)
                tt(D[s][:, 0:g, 4:8, :], SQP[s][:, 0:g], SQL[s][:, 0:g],
                   Alu.subtract).then_inc(ds_done, 1)
                v.drain()
                tt(OV[s][:, 0:g], SM2[s][:, 0:g, 4:8, :], TA[s][:, 0:g],
                   Alu.min)
                if fills:
                    cls_fill(*fills.pop(0))
                v.drain()
                ts(CLt[s][:, 0:g], OV[s][:, 0:g], 0.0, None, Alu.max)
                if fills:
                    cls_fill(*fills.pop(0))
                v.drain()
                clv = CLt[s][:, 0:g].rearrange("p g (b c) k -> p g b c k",
                                               b=2)
                tt(INT[s][:, 0:g], clv[:, :, :, 0, :], clv[:, :, :, 1, :],
                   Alu.mult)
                if fills:
                    cls_fill(*fills.pop(0))
                v.drain()
                ts(SA49[s][:, 0:g], SA[s][:, 0:g], 49.0, None, Alu.mult)
                if fills:
                    cls_fill(*fills.pop(0))
                v.drain()
                tt(U49[s][:, 0:g], SA49[s][:, 0:g], INT[s][:, 0:g],
                   Alu.subtract)
                v.drain()
                with nc.allow_low_precision(reason="bf16 iou ok"):
                    v.reciprocal(RCPt[s][:, 0:g], U49[s][:, 0:g])
                v.drain()
                tt(IOU[s][:, 0:g], INT[s][:, 0:g], RCPt[s][:, 0:g], Alu.mult)
                v.drain()
                tt(Ut[s][:, 0:g], IOU[s][:, 0:g, 0:1, :],
                   IOU[s][:, 0:g, 1:2, :], Alu.is_ge)
                tt(D[s][:, 0:g, 8:10, :], p[:, 0:g, 8:10, :], IOU[s][:, 0:g],
                   Alu.subtract)
                v.drain().then_inc(u_done, 1)
                if cfg["sq_coor"] == "dve":
                    tt(SQ[s][:, 0:g, 0:8, :], D[s][:, 0:g, 0:8, :],
                       D[s][:, 0:g, 0:8, :], Alu.mult).then_inc(sqA, 1)
                if not last:
                    while fills:
                        cls_fill(*fills.pop(0))
                    v.drain().then_inc(dve_in, 1)
                else:
                    # last chunk: rm build on DVE (shortest tail)
                    v.wait_ge(obj_done, i + 1)
                    ts(TQ[s][:, 0:g, 0:1, :], Ut[s][:, 0:g],
                       0.5, 0.5, Alu.mult, Alu.add)
                    ts(TQ[s][:, 0:g, 1:2, :], Ut[s][:, 0:g],
                       -0.5, 1.0, Alu.mult, Alu.add)
                    ts(O5[s][:, 0:g], WT[s][:, 0:g, 8:10, :],
                       5.0, None, Alu.mult)
                    v.drain().then_inc(dve_in, 1)
                    tt(WT[s][:, 0:g, 0:2, :], O5[s][:, 0:g],
                       Ut[s][:, 0:g].broadcast_to([P, g, 2, K]), Alu.mult)
                    tt(WT[s][:, 0:g, 4:6, :], TQ[s][:, 0:g],
                       WT[s][:, 0:g, 8:10, :], Alu.mult)
                    v.drain()
                    tt(WT[s][:, 0:g, 2:4, :], O5[s][:, 0:g],
                       WT[s][:, 0:g, 0:2, :],
                       Alu.subtract).then_inc(rm_done, 1)
                    v.drain()
                    while fills:
                        cls_fill(*fills.pop(0))
                    v.drain()

        @block.tensor
        def _(pe):
            first = True
            for i, g in enumerate(chunks):
                s = i % 2

                def mm(c, w, gi, last=False):
                    nonlocal first
                    r = pe.matmul(
                        ps[0:98, :],
                        WT[s][:, gi, w:w + 2, :].rearrange(
                            "p c k -> p (c k)"),
                        SQ[s][:, gi, c:c + 2, :].rearrange(
                            "p c k -> p (c k)"),
                        start=first, stop=last, skip_group_check=True)
                    first = False
                    return r

                # wave B: cls blocks per gi (squares stream in per g-unit)
                for gi in range(g):
                    pe.wait_ge(clsq, gbase[i] + gi + 1)
                    if gi == 0:
                        pe.wait_ge(rm4_done, i + 1)
                    for c in range(10, 30, 2):
                        mm(c, 8, gi)
                # wave A: coor + conf + noobj per chunk
                pe.wait_ge(sqA, i + 1)
                pe.wait_ge(rm_done, i + 1)
                for gi in range(g):
                    mm(0, 0, gi)
                    mm(2, 2, gi)
                    mm(4, 0, gi)
                    mm(6, 2, gi)
                    mm(8, 4, gi)
                    lastmm = (i == nchunk - 1 and gi == g - 1)
                    r = mm(30, 6, gi, last=lastmm)
                    if gi == g - 1:
                        r.then_inc(pe_done, 1)

    return nc


_NC_CACHE = {}


def _get_nc():
    if "nc" not in _NC_CACHE:
        _NC_CACHE["nc"] = build_nc()
    return _NC_CACHE["nc"]


def _to_bf16_repack(pred, labels):
    import ml_dtypes

    bf = ml_dtypes.bfloat16
    p = np.ascontiguousarray(pred, dtype=np.float32).reshape(B_TOTAL, 30, K)
    l = np.ascontiguousarray(labels, dtype=np.float32).reshape(B_TOTAL, 30, K)
    pb = p.astype(bf)
    lb = l.astype(bf)
    # obj channel: keep the ==1.0 test exact under rounding
    l4 = l[:, 4, :]
    lb4 = lb[:, 4, :]
    bad = (l4 != np.float32(1.0)) & (lb4.astype(np.float32) == np.float32(1.0))
    if bad.any():
        lb4[bad] = bf(0.99609375)
        lb[:, 4, :] = lb4
    xall = np.concatenate(
        [pb[:, PP_IDX[:PH], :], lb[:, LL_IDX[:LH], :],
         pb[:, PP_IDX[PH:], :], lb[:, LL_IDX[LH:], :]], axis=1)
    return np.ascontiguousarray(xall).reshape(B_TOTAL, ROWX)


def run_device(pred, labels, trace=False):
    from concourse.bass_utils import run_bass_kernel_spmd

    nc = _get_nc()
    xrp = _to_bf16_repack(pred, labels)
    in_maps = []
    for c in range(NCORES):
        rows = slice(c * B_CORE, (c + 1) * B_CORE)
        in_maps.append({"x": xrp[rows]})
    res = run_bass_kernel_spmd(nc, in_maps, list(range(NCORES)), trace=trace)
    total = 0.0
    for c in range(NCORES):
        m = res.results[c]["out2"].astype(np.float64)
        total += float(np.trace(m[0:98, 0:98]))
    loss = np.float32(total / B_TOTAL)
    return loss, res


def kernel(pred, labels):
    loss, _ = run_device(pred, labels, trace=False)
    return np.array(loss, dtype=np.float32)


if __name__ == "__main__":
    rng = np.random.default_rng(0)
    p = rng.random((B_TOTAL, 30, 7, 7), dtype=np.float32)
    l = rng.random((B_TOTAL, 30, 7, 7), dtype=np.float32)
    l[:, 4] = (rng.random((B_TOTAL, 7, 7)) < 0.3).astype(np.float32)
    print(kernel(p, l))


# revision 4
# speedup vs baseline: 1.0153x; 1.0011x over previous
"""YOLOv1 loss kernel v2 for Trainium2, 8-core data-parallel, bf16.

Layout per core (1024 rows, chunks of g*128 rows, tiles [128, g, ch, 49]):
  pred  PP (30 ch): [x1,y1,x2,y2, w1,h1,w2,h2, c1,c2, cls*20]
  label LL (29 ch): [gx,gy, x5,y5, gw,gh, w7,h7, obj, cls*20]

Math: iou via 7x-scaled overlap ov = relu(min(7*min(w,wg), 3.5*(w+wg)-|dc|)),
ints = ovx*ovy, u49 = 49*(areas)-ints, iou = ints/u49.
All loss terms become sum_cells w_ch * d_ch^2 with per-cell weights:
  coor (D 0:8 = [c1x,c1y,c2x,c2y, s1w,s1h,s2w,s2h]): w = 5*obj*u / 5*obj*(1-u)
  conf (D 8:10 = dconf): w = obj*(.5+.5u) / obj*(1-.5u)
  cls  (D 10:30): w = obj
  noobj (pred conf raw, squared into SQ 30:32): w = .5*(1-obj)
Squares SQ = D^2 (Act; DVE self-mult on the last chunk); the weighted
accumulation runs on the idle PE: per 2-channel block,
matmul(psum[0:98,0:98], lhsT=weight-broadcast, rhs=SQ-block) accumulated over
all blocks/chunks; diag(psum)[i] = sum_p w[p,i]*sq[p,i]. Host sums the psum
diagonal (f64) -> loss.

Engines: DVE = diffs/iou pipeline; Pool = obj/weight builds/cls share;
Act = sqrt(5*w), squares; PE = weighted accumulation; SP = DMA.
cls diffs/squares pipeline per g-unit so PE streams behind Act.
"""

import sys

import numpy as np

for _p in ("/opt/trn_rl_repo", "/root/.axon_site/_ro/trn_rl_repo"):
    if _p not in sys.path:
        sys.path.insert(0, _p)

import concourse.bass as bass
import concourse.mybir as mybir

F32 = mybir.dt.float32
BF16 = mybir.dt.bfloat16
U16 = mybir.dt.uint16
Alu = mybir.AluOpType
Act = mybir.ActivationFunctionType

B_TOTAL = 8192
NCORES = 8
B_CORE = B_TOTAL // NCORES  # 1024
P = 128
K = 49
CP = 30
CL = 29
ROWP = CP * K
ROWL = CL * K

PP_IDX = [0, 1, 5, 6, 2, 3, 7, 8, 4, 9] + list(range(10, 30))
LL_IDX = [0, 1, 5, 6, 2, 3, 7, 8, 4] + list(range(10, 30))

PH = 10
LH = 9
CX = CP + CL          # 59 combined channels per row
HX = PH + LH          # 19 head channels
ROWX = CX * K

CFG = dict(
    chunks=(2, 3, 3),
    cls_pool=10,      # cls diff channels done on Pool (rest on DVE)
    sq_coor="act",    # coor squares on act or dve
    act_split=False,  # conf square separate from coor square on Act
)


def build_nc(cfg=None):
    cfg = dict(CFG, **(cfg or {}))
    chunks = cfg["chunks"]
    clsp = cfg["cls_pool"]
    clsd = 20 - clsp
    assert sum(chunks) * P == B_CORE
    nchunk = len(chunks)
    ilast = nchunk - 1
    maxg = max(chunks)
    nc = bass.Bass()
    x = nc.declare_dram_parameter("x", [B_CORE, ROWX], BF16, isOutput=False)
    out2 = nc.declare_dram_parameter("out2", [P, 98], F32, isOutput=True)

    from contextlib import ExitStack

    _ctr = [0]
    es = ExitStack()

    def sb(shape, dt=BF16):
        _ctr[0] += 1
        return es.enter_context(nc.sbuf_tensor(f"t{_ctr[0]}", shape, dt))

    with es:
        # combined tiles: xh = head (pred 0:10 | label 0:9),
        # xtl = tail (pred cls 20 | label cls 20)
        xh = [sb([P, maxg, HX, K]) for _ in range(2)]
        xtl = [sb([P, maxg, 40, K]) for _ in range(2)]
        XT = [sb([P, maxg, 4, K]) for _ in range(2)]
        AD = [sb([P, maxg, 4, K]) for _ in range(2)]
        SM = [sb([P, maxg, 8, K]) for _ in range(2)]
        SM2 = [sb([P, maxg, 8, K]) for _ in range(2)]
        TA = [sb([P, maxg, 4, K]) for _ in range(2)]
        OV = [sb([P, maxg, 4, K]) for _ in range(2)]
        CLt = [sb([P, maxg, 4, K]) for _ in range(2)]
        AR = [sb([P, maxg, 3, K]) for _ in range(2)]
        SA = [sb([P, maxg, 2, K]) for _ in range(2)]
        SA49 = [sb([P, maxg, 2, K]) for _ in range(2)]
        INT = [sb([P, maxg, 2, K]) for _ in range(2)]
        U49 = [sb([P, maxg, 2, K]) for _ in range(2)]
        RCPt = [sb([P, maxg, 2, K]) for _ in range(2)]
        IOU = [sb([P, maxg, 2, K]) for _ in range(2)]
        Ut = [sb([P, maxg, 1, K]) for _ in range(2)]
        O5 = [sb([P, maxg, 2, K]) for _ in range(2)]
        TQ = [sb([P, maxg, 2, K]) for _ in range(2)]
        # weight tile: [rm0,rm0 | rm1,rm1 | rm2,rm3 | rm4,rm4 | obj,obj]
        WT = [sb([P, maxg, 10, K]) for _ in range(2)]
        SQP = [sb([P, maxg, 4, K]) for _ in range(2)]
        SQL = [sb([P, maxg, 4, K]) for _ in range(2)]
        D = [sb([P, maxg, 30, K]) for _ in range(2)]
        SQ = [sb([P, maxg, 32, K]) for _ in range(2)]
        PSB = es.enter_context(nc.sbuf_tensor("psb", [P, 98], F32))
        ps = es.enter_context(nc.psum_tensor("ps", [P, 98], F32))

        sem_names = ["dsemHA", "dsemHB", "dsemTA", "dsemTB", "u_done",
                     "sqrt_done", "ds_done", "clsq", "rm_done", "sqA",
                     "pe_done", "dve_in", "act_in", "pool_in",
                     "cls_dve_g", "cls_pool_g", "obj_done", "rm4_done",
                     "sa_done", "dsemTL", "clsqP", "psb_done", "outd"]
        sems = {n: es.enter_context(nc.semaphore(n)) for n in sem_names}
        dsemH = [sems["dsemHA"], sems["dsemHB"]]
        dsemT = [sems["dsemTA"], sems["dsemTB"]]
        u_done = sems["u_done"]
        sqrt_done = sems["sqrt_done"]
        ds_done = sems["ds_done"]
        clsq = sems["clsq"]          # cls squares done, +1 per (chunk, gi)
        rm_done = sems["rm_done"]
        sqA = sems["sqA"]            # coor+conf+noobj squares, +1 per chunk
        pe_done = sems["pe_done"]
        dve_in = sems["dve_in"]
        act_in = sems["act_in"]
        pool_in = sems["pool_in"]
        cls_dve_g = sems["cls_dve_g"]    # cls diffs (dve share) per (chunk,gi)
        cls_pool_g = sems["cls_pool_g"]  # cls diffs (pool share) per (chunk,gi)
        obj_done = sems["obj_done"]
        rm4_done = sems["rm4_done"]
        sa_done = sems["sa_done"]
        dsemTL = sems["dsemTL"]
        clsqP = sems["clsqP"]
        psb_done = sems["psb_done"]
        outd = sems["outd"]

        block = es.enter_context(nc.Block())

        offs = [0]
        for g in chunks:
            offs.append(offs[-1] + g * P)

        headv = {}
        tailv = {}
        _hv = [0, 0]
        _tv = [0, 0]
        for i in range(nchunk):
            s = i % 2
            _hv[s] += 32
            _tv[s] += 32
            headv[i] = _hv[s]
            tailv[i] = _tv[s]
        # cumulative (chunk, gi) counter base: number of g-units before chunk i
        gbase = [sum(chunks[:i]) for i in range(nchunk)]

        @block.sync
        def _(sync):
            def head(i):
                g = chunks[i]
                s = i % 2
                if i >= 2:
                    # xh[s] readers of chunk i-2: dve dconf (<= u_done),
                    # act sqno (act_in), pool obj (obj_done)
                    sync.wait_ge(u_done, i - 1)
                    sync.wait_ge(act_in, i - 1)
                    sync.wait_ge(obj_done, i - 1)
                rows = slice(offs[i], offs[i + 1])
                sync.dma_start(
                    out=xh[s][:, 0:g].rearrange("p g c k -> p g (c k)"),
                    in_=x[rows, 0:HX * K].rearrange("(g p) d -> p g d", p=P),
                ).then_inc(dsemH[s], 32)

            def tail(i):
                g = chunks[i]
                s = i % 2
                if i >= 2:
                    # xtl[s] readers of chunk i-2: dve + pool cls diffs
                    sync.wait_ge(cls_dve_g, gbase[i - 1])
                    if clsp > 0:
                        sync.wait_ge(cls_pool_g, gbase[i - 1])
                rows = slice(offs[i], offs[i + 1])
                if i == ilast:
                    for gi in range(g):
                        rg = slice(offs[i] + gi * P, offs[i] + (gi + 1) * P)
                        sync.dma_start(
                            out=xtl[s][:, gi].rearrange("p c k -> p (c k)"),
                            in_=x[rg, HX * K:].rearrange(
                                "(g p) d -> p (g d)", p=P),
                        ).then_inc(dsemTL, 16)
                else:
                    sync.dma_start(
                        out=xtl[s][:, 0:g].rearrange("p g c k -> p g (c k)"),
                        in_=x[rows, HX * K:].rearrange("(g p) d -> p g d", p=P),
                    ).then_inc(dsemT[s], 32)

            for i in range(nchunk):
                head(i)
                tail(i)
            sync.wait_ge(outd, 16)

        @block.gpsimd
        def _(gp):
            for i, g in enumerate(chunks):
                s = i % 2
                if i >= 2:
                    gp.wait_ge(pe_done, i - 1)
                gp.wait_ge(dsemH[s], headv[i])
                gp.tensor_scalar(WT[s][:, 0:g, 8:10, :],
                                 xh[s][:, 0:g, 18:19, :].broadcast_to(
                                     [P, g, 2, K]),
                                 1.0, None,
                                 Alu.is_equal).then_inc(obj_done, 1)
                gp.drain()
                gp.tensor_scalar(WT[s][:, 0:g, 6:8, :], WT[s][:, 0:g, 8:10, :],
                                 -0.5, 0.5, Alu.mult,
                                 Alu.add).then_inc(rm4_done, 1)
                gp.drain().then_inc(pool_in, 1)

                def pool_cls(j):
                    sj = j % 2
                    if clsp == 0:
                        return
                    if j >= 2:
                        gp.wait_ge(sqA, j - 1)
                        gp.wait_ge(clsq, gbase[j - 1])
                        gp.wait_ge(clsqP, gbase[j - 1])
                    for gi in range(chunks[j]):
                        if j == ilast:
                            gp.wait_ge(dsemTL, 16 * (gi + 1))
                        elif gi == 0:
                            gp.wait_ge(dsemT[sj], tailv[j])
                        gp.tensor_tensor(
                            D[sj][:, gi, 10 + clsd:30, :],
                            xtl[sj][:, gi, clsd:20, :],
                            xtl[sj][:, gi, 20 + clsd:40, :], Alu.subtract,
                        ).then_inc(cls_pool_g, 1)
                    gp.drain()

                pool_cls(i)
                if i == ilast:
                    continue  # last-chunk rm built on DVE
                gp.wait_ge(u_done, i + 1)
                gp.tensor_scalar(O5[s][:, 0:g], WT[s][:, 0:g, 8:10, :],
                                 5.0, None, Alu.mult)
                gp.drain()
                gp.tensor_tensor(WT[s][:, 0:g, 0:2, :], O5[s][:, 0:g],
                                 Ut[s][:, 0:g].broadcast_to([P, g, 2, K]),
                                 Alu.mult)
                gp.tensor_scalar(TQ[s][:, 0:g, 0:1, :], Ut[s][:, 0:g],
                                 0.5, 0.5, Alu.mult, Alu.add)
                gp.tensor_scalar(TQ[s][:, 0:g, 1:2, :], Ut[s][:, 0:g],
                                 -0.5, 1.0, Alu.mult, Alu.add)
                gp.drain()
                gp.tensor_tensor(WT[s][:, 0:g, 2:4, :], O5[s][:, 0:g],
                                 WT[s][:, 0:g, 0:2, :], Alu.subtract)
                gp.tensor_tensor(
                    WT[s][:, 0:g, 4:6, :], TQ[s][:, 0:g],
                    WT[s][:, 0:g, 8:10, :], Alu.mult,
                ).then_inc(rm_done, 1)
                gp.drain()

        @block.scalar
        def _(act):
            for i, g in enumerate(chunks):
                s = i % 2
                if i >= 2:
                    act.wait_ge(pe_done, i - 1)   # SQ reuse
                    act.wait_ge(ds_done, i - 1)   # SQP/SQL reuse
                act.wait_ge(dsemH[s], headv[i])
                act.activation(SQP[s][:, 0:g], xh[s][:, 0:g, 4:8, :],
                               Act.Sqrt)
                act.activation(SQL[s][:, 0:g], xh[s][:, 0:g, 14:18, :],
                               Act.Sqrt).then_inc(sqrt_done, 1)
                act.activation(SQ[s][:, 0:g, 30:32, :],
                               xh[s][:, 0:g, 8:10, :],
                               Act.Square).then_inc(act_in, 1)
                for gi in range(g):
                    act.wait_ge(cls_dve_g, gbase[i] + gi + 1)
                    act.activation(SQ[s][:, gi, 10:10 + clsd, :],
                                   D[s][:, gi, 10:10 + clsd, :],
                                   Act.Square).then_inc(clsq, 1)
                for gi in range(g):
                    act.wait_ge(cls_pool_g, gbase[i] + gi + 1)
                    act.activation(SQ[s][:, gi, 10 + clsd:30, :],
                                   D[s][:, gi, 10 + clsd:30, :],
                                   Act.Square).then_inc(clsqP, 1)
                act.wait_ge(u_done, i + 1)
                act.wait_ge(ds_done, i + 1)
                act.activation(SQ[s][:, 0:g, 0:10, :],
                               D[s][:, 0:g, 0:10, :],
                               Act.Square).then_inc(sqA, 1)
            act.wait_ge(pe_done, nchunk)
            act.activation(PSB[0:98, :], ps[0:98, :],
                           Act.Copy).then_inc(psb_done, 1)
            act.dma_start(out=out2[:], in_=PSB[:]).then_inc(outd, 16)

        @block.vector
        def _(v):
            tt = v.tensor_tensor
            ts = v.tensor_scalar

            def cls_fill(j, gi):
                """emit chunk j's cls diff for unit gi (waits its tail dma)"""
                sj = j % 2
                if j == ilast:
                    v.wait_ge(dsemTL, 16 * (gi + 1))
                elif gi == 0:
                    v.wait_ge(dsemT[sj], tailv[j])
                tt(D[sj][:, gi, 10:10 + clsd, :],
                   xtl[sj][:, gi, 0:clsd, :],
                   xtl[sj][:, gi, 20:20 + clsd, :],
                   Alu.subtract).then_inc(cls_dve_g, 1)

            for i, g in enumerate(chunks):
                s = i % 2
                last = (i == ilast)
                p = xh[s]
                # cls fills ride in this chunk's own chain drain slots
                fills = [] if last else [(i, gi) for gi in range(g)]
                if i >= 2:
                    v.wait_ge(sqA, i - 1)      # D reuse (act read done)
                    v.wait_ge(clsq, gbase[i - 1])  # prev-parity cls read
                    v.wait_ge(clsqP, gbase[i - 1])
                    v.wait_ge(rm_done, i - 1)  # Ut reuse
                    if cfg["sq_coor"] == "dve":
                        v.wait_ge(pe_done, i - 1)  # SQ reuse
                if last:
                    # cls diffs first: tail DMA streams while the previous
                    # chunk still computes; Act/PE stream cls early
                    for gi in range(g):
                        cls_fill(i, gi)
                v.wait_ge(dsemH[s], headv[i])
                gxy2 = (p[:, 0:g, 10:12, :].unsqueeze(2)
                        .broadcast_to([P, g, 2, 2, K]))
                gwh2 = (p[:, 0:g, 14:16, :].unsqueeze(2)
                        .broadcast_to([P, g, 2, 2, K]))
                pxy_v = p[:, 0:g, 0:4, :].rearrange(
                    "p g (b c) k -> p g b c k", b=2)
                pwh_v = p[:, 0:g, 4:8, :].rearrange(
                    "p g (b c) k -> p g b c k", b=2)
                # W1
                tt(XT[s][:, 0:g].rearrange("p g (b c) k -> p g b c k", b=2),
                   pxy_v, gxy2, Alu.subtract)
                tt(D[s][:, 0:g, 2:4, :], p[:, 0:g, 2:4, :],
                   p[:, 0:g, 12:14, :], Alu.subtract)
                tt(SM[s][:, 0:g, 0:4, :].rearrange(
                    "p g (b c) k -> p g b c k", b=2),
                   pwh_v, gwh2, Alu.add)
                tt(SM[s][:, 0:g, 4:8, :].rearrange(
                    "p g (b c) k -> p g b c k", b=2),
                   pwh_v, gwh2, Alu.min)
                tt(AR[s][:, 0:g, 0:2, :],
                   pwh_v[:, :, :, 0, :], pwh_v[:, :, :, 1, :], Alu.mult)
                tt(AR[s][:, 0:g, 2:3, :], p[:, 0:g, 14:15, :],
                   p[:, 0:g, 15:16, :], Alu.mult)
                v.drain()
                # W1b
                v.tensor_copy(D[s][:, 0:g, 0:2, :], XT[s][:, 0:g, 0:2, :])
                ts(AD[s][:, 0:g].bitcast(U16), XT[s][:, 0:g].bitcast(U16),
                   0x7FFF, None, Alu.bitwise_and)
                ts(SM2[s][:, 0:g, 0:4, :], SM[s][:, 0:g, 0:4, :],
                   3.5, None, Alu.mult)
                ts(SM2[s][:, 0:g, 4:8, :], SM[s][:, 0:g, 4:8, :],
                   7.0, None, Alu.mult)
                tt(SA[s][:, 0:g], AR[s][:, 0:g, 0:2, :],
                   AR[s][:, 0:g, 2:3, :].broadcast_to([P, g, 2, K]),
                   Alu.add)
                v.drain()
                # W2 chain; prev chunk's cls diffs + ds fill the drain slots
                tt(TA[s][:, 0:g], SM2[s][:, 0:g, 0:4, :], AD[s][:, 0:g],
                   Alu.subtract)
                v.wait_ge(sqrt_done, i + 1)
                tt(D[s][:, 0:g, 4:8, :], SQP[s][:, 0:g], SQL[s][:, 0:g],
                   Alu.subtract).then_inc(ds_done, 1)
                v.drain()
                tt(OV[s][:, 0:g], SM2[s][:, 0:g, 4:8, :], TA[s][:, 0:g],
                   Alu.min)
                if fills:
                    cls_fill(*fills.pop(0))
                v.drain()
                ts(CLt[s][:, 0:g], OV[s][:, 0:g], 0.0, None, Alu.max)
                if fills:
                    cls_fill(*fills.pop(0))
                v.drain()
                clv = CLt[s][:, 0:g].rearrange("p g (b c) k -> p g b c k",
                                               b=2)
                tt(INT[s][:, 0:g], clv[:, :, :, 0, :], clv[:, :, :, 1, :],
                   Alu.mult)
                if fills:
                    cls_fill(*fills.pop(0))
                v.drain()
                ts(SA49[s][:, 0:g], SA[s][:, 0:g], 49.0, None, Alu.mult)
                if fills:
                    cls_fill(*fills.pop(0))
                v.drain()
                tt(U49[s][:, 0:g], SA49[s][:, 0:g], INT[s][:, 0:g],
                   Alu.subtract)
                v.drain()
                with nc.allow_low_precision(reason="bf16 iou ok"):
                    v.reciprocal(RCPt[s][:, 0:g], U49[s][:, 0:g])
                v.drain()
                tt(IOU[s][:, 0:g], INT[s][:, 0:g], RCPt[s][:, 0:g], Alu.mult)
                v.drain()
                tt(Ut[s][:, 0:g], IOU[s][:, 0:g, 0:1, :],
                   IOU[s][:, 0:g, 1:2, :], Alu.is_ge)
                tt(D[s][:, 0:g, 8:10, :], p[:, 0:g, 8:10, :], IOU[s][:, 0:g],
                   Alu.subtract)
                v.drain().then_inc(u_done, 1)
                if cfg["sq_coor"] == "dve":
                    tt(SQ[s][:, 0:g, 0:8, :], D[s][:, 0:g, 0:8, :],
                       D[s][:, 0:g, 0:8, :], Alu.mult).then_inc(sqA, 1)
                if not last:
                    while fills:
                        cls_fill(*fills.pop(0))
                    v.drain().then_inc(dve_in, 1)
                else:
                    # last chunk: rm build on DVE (shortest tail)
                    v.wait_ge(obj_done, i + 1)
                    ts(TQ[s][:, 0:g, 0:1, :], Ut[s][:, 0:g],
                       0.5, 0.5, Alu.mult, Alu.add)
                    ts(TQ[s][:, 0:g, 1:2, :], Ut[s][:, 0:g],
                       -0.5, 1.0, Alu.mult, Alu.add)
                    ts(O5[s][:, 0:g], WT[s][:, 0:g, 8:10, :],
                       5.0, None, Alu.mult)
                    v.drain().then_inc(dve_in, 1)
                    tt(WT[s][:, 0:g, 0:2, :], O5[s][:, 0:g],
                       Ut[s][:, 0:g].broadcast_to([P, g, 2, K]), Alu.mult)
                    tt(WT[s][:, 0:g, 4:6, :], TQ[s][:, 0:g],
                       WT[s][:, 0:g, 8:10, :], Alu.mult)
                    v.drain()
                    tt(WT[s][:, 0:g, 2:4, :], O5[s][:, 0:g],
                       WT[s][:, 0:g, 0:2, :],
                       Alu.subtract).then_inc(rm_done, 1)
                    v.drain()
                    while fills:
                        cls_fill(*fills.pop(0))
                    v.drain()

        @block.tensor
        def _(pe):
            first = True
            for i, g in enumerate(chunks):
                s = i % 2

                def mm(c, w, gi, last=False):
                    nonlocal first
                    r = pe.matmul(
                        ps[0:98, :],
                        WT[s][:, gi, w:w + 2, :].rearrange(
                            "p c k -> p (c k)"),
                        SQ[s][:, gi, c:c + 2, :].rearrange(
                            "p c k -> p (c k)"),
                        start=first, stop=last, skip_group_check=True)
                    first = False
                    return r

                # wave B: cls blocks; dve-share pairs stream first
                for gi in range(g):
                    pe.wait_ge(clsq, gbase[i] + gi + 1)
                    if gi == 0:
                        pe.wait_ge(rm4_done, i + 1)
                    for c in range(10, 10 + clsd, 2):
                        mm(c, 8, gi)
                for gi in range(g):
                    pe.wait_ge(clsqP, gbase[i] + gi + 1)
                    for c in range(10 + clsd, 30, 2):
                        mm(c, 8, gi)
                # wave A: coor + conf + noobj per chunk
                pe.wait_ge(sqA, i + 1)
                pe.wait_ge(rm_done, i + 1)
                for gi in range(g):
                    mm(0, 0, gi)
                    mm(2, 2, gi)
                    mm(4, 0, gi)
                    mm(6, 2, gi)
                    mm(8, 4, gi)
                    lastmm = (i == nchunk - 1 and gi == g - 1)
                    r = mm(30, 6, gi, last=lastmm)
                    if gi == g - 1:
                        r.then_inc(pe_done, 1)

    return nc


_NC_CACHE = {}


def _get_nc():
    if "nc" not in _NC_CACHE:
        _NC_CACHE["nc"] = build_nc()
    return _NC_CACHE["nc"]


def _to_bf16_repack(pred, labels):
    import ml_dtypes

    bf = ml_dtypes.bfloat16
    p = np.ascontiguousarray(pred, dtype=np.float32).reshape(B_TOTAL, 30, K)
    l = np.ascontiguousarray(labels, dtype=np.float32).reshape(B_TOTAL, 30, K)
    pb = p.astype(bf)
    lb = l.astype(bf)
    # obj channel: keep the ==1.0 test exact under rounding
    l4 = l[:, 4, :]
    lb4 = lb[:, 4, :]
    bad = (l4 != np.float32(1.0)) & (lb4.astype(np.float32) == np.float32(1.0))
    if bad.any():
        lb4[bad] = bf(0.99609375)
        lb[:, 4, :] = lb4
    xall = np.concatenate(
        [pb[:, PP_IDX[:PH], :], lb[:, LL_IDX[:LH], :],
         pb[:, PP_IDX[PH:], :], lb[:, LL_IDX[LH:], :]], axis=1)
    return np.ascontiguousarray(xall).reshape(B_TOTAL, ROWX)


def run_device(pred, labels, trace=False):
    from concourse.bass_utils import run_bass_kernel_spmd

    nc = _get_nc()
    xrp = _to_bf16_repack(pred, labels)
    in_maps = []
    for c in range(NCORES):
        rows = slice(c * B_CORE, (c + 1) * B_CORE)
        in_maps.append({"x": xrp[rows]})
    res = run_bass_kernel_spmd(nc, in_maps, list(range(NCORES)), trace=trace)
    total = 0.0
    for c in range(NCORES):
        m = res.results[c]["out2"].astype(np.float64)
        total += float(np.trace(m[0:98, 0:98]))
    loss = np.float32(total / B_TOTAL)
    return loss, res


def kernel(pred, labels):
    loss, _ = run_device(pred, labels, trace=False)
    return np.array(loss, dtype=np.float32)


if __name__ == "__main__":
    rng = np.random.default_rng(0)
    p = rng.random((B_TOTAL, 30, 7, 7), dtype=np.float32)
    l = rng.random((B_TOTAL, 30, 7, 7), dtype=np.float32)
    l[:, 4] = (rng.random((B_TOTAL, 7, 7)) < 0.3).astype(np.float32)
    print(kernel(p, l))


# revision 5
# speedup vs baseline: 1.0327x; 1.0172x over previous
"""YOLOv1 loss kernel v2 for Trainium2, 8-core data-parallel, bf16.

Layout per core (1024 rows, chunks of g*128 rows, tiles [128, g, ch, 49]):
  pred  PP (30 ch): [x1,y1,x2,y2, w1,h1,w2,h2, c1,c2, cls*20]
  label LL (29 ch): [gx,gy, x5,y5, gw,gh, w7,h7, obj, cls*20]

Math: iou via 7x-scaled overlap ov = relu(min(7*min(w,wg), 3.5*(w+wg)-|dc|)),
ints = ovx*ovy, u49 = 49*(areas)-ints, iou = ints/u49.
All loss terms become sum_cells w_ch * d_ch^2 with per-cell weights:
  coor (D 0:8 = [c1x,c1y,c2x,c2y, s1w,s1h,s2w,s2h]): w = 5*obj*u / 5*obj*(1-u)
  conf (D 8:10 = dconf): w = obj*(.5+.5u) / obj*(1-.5u)
  cls  (D 10:30): w = obj
  noobj (pred conf raw, squared into SQ 30:32): w = .5*(1-obj)
Squares SQ = D^2 (Act; DVE self-mult on the last chunk); the weighted
accumulation runs on the idle PE: per 2-channel block,
matmul(psum[0:98,0:98], lhsT=weight-broadcast, rhs=SQ-block) accumulated over
all blocks/chunks; diag(psum)[i] = sum_p w[p,i]*sq[p,i]. Host sums the psum
diagonal (f64) -> loss.

Engines: DVE = diffs/iou pipeline; Pool = obj/weight builds/cls share;
Act = sqrt(5*w), squares; PE = weighted accumulation; SP = DMA.
cls diffs/squares pipeline per g-unit so PE streams behind Act.
"""

import sys

import numpy as np

for _p in ("/opt/trn_rl_repo", "/root/.axon_site/_ro/trn_rl_repo"):
    if _p not in sys.path:
        sys.path.insert(0, _p)

import concourse.bass as bass
import concourse.mybir as mybir

F32 = mybir.dt.float32
BF16 = mybir.dt.bfloat16
U16 = mybir.dt.uint16
Alu = mybir.AluOpType
Act = mybir.ActivationFunctionType

B_TOTAL = 8192
NCORES = 8
B_CORE = B_TOTAL // NCORES  # 1024
P = 128
K = 49
CP = 30
CL = 29
ROWP = CP * K
ROWL = CL * K

PP_IDX = [0, 1, 5, 6, 2, 3, 7, 8, 4, 9] + list(range(10, 30))
LL_IDX = [0, 1, 5, 6, 2, 3, 7, 8, 4] + list(range(10, 30))

PH = 10
LH = 9
CX = CP + CL          # 59 combined channels per row
HX = PH + LH          # 19 head channels
ROWX = CX * K

CFG = dict(
    chunks=(2, 3, 3),
    cls_pool=10,      # cls diff channels done on Pool (rest on DVE)
    sq_coor="act",    # coor squares on act or dve
    act_split=False,  # conf square separate from coor square on Act
)


def build_nc(cfg=None):
    cfg = dict(CFG, **(cfg or {}))
    chunks = cfg["chunks"]
    clsp = cfg["cls_pool"]
    clsd = 20 - clsp
    assert sum(chunks) * P == B_CORE
    nchunk = len(chunks)
    ilast = nchunk - 1
    maxg = max(chunks)
    nc = bass.Bass()
    x = nc.declare_dram_parameter("x", [B_CORE, ROWX], BF16, isOutput=False)
    out2 = nc.declare_dram_parameter("out2", [P, 98], F32, isOutput=True)

    from contextlib import ExitStack

    _ctr = [0]
    es = ExitStack()

    def sb(shape, dt=BF16):
        _ctr[0] += 1
        return es.enter_context(nc.sbuf_tensor(f"t{_ctr[0]}", shape, dt))

    with es:
        # combined tiles: xh = head (pred 0:10 | label 0:9),
        # xtl = tail (pred cls 20 | label cls 20)
        xh = [sb([P, maxg, HX, K]) for _ in range(2)]
        xtl = [sb([P, maxg, 40, K]) for _ in range(2)]
        XT = [sb([P, maxg, 4, K]) for _ in range(2)]
        AD = [sb([P, maxg, 4, K]) for _ in range(2)]
        SM = [sb([P, maxg, 8, K]) for _ in range(2)]
        SM2 = [sb([P, maxg, 8, K]) for _ in range(2)]
        TA = [sb([P, maxg, 4, K]) for _ in range(2)]
        OV = [sb([P, maxg, 4, K]) for _ in range(2)]
        CLt = [sb([P, maxg, 4, K]) for _ in range(2)]
        AR = [sb([P, maxg, 3, K]) for _ in range(2)]
        SA = [sb([P, maxg, 2, K]) for _ in range(2)]
        SA49 = [sb([P, maxg, 2, K]) for _ in range(2)]
        INT = [sb([P, maxg, 2, K]) for _ in range(2)]
        U49 = [sb([P, maxg, 2, K]) for _ in range(2)]
        RCPt = [sb([P, maxg, 2, K]) for _ in range(2)]
        IOU = [sb([P, maxg, 2, K]) for _ in range(2)]
        Ut = [sb([P, maxg, 1, K]) for _ in range(2)]
        O5 = [sb([P, maxg, 2, K]) for _ in range(2)]
        TQ = [sb([P, maxg, 2, K]) for _ in range(2)]
        # weight tile: [rm0,rm0 | rm1,rm1 | rm2,rm3 | rm4,rm4 | obj,obj]
        WT = [sb([P, maxg, 10, K]) for _ in range(2)]
        SQP = [sb([P, maxg, 4, K]) for _ in range(2)]
        SQL = [sb([P, maxg, 4, K]) for _ in range(2)]
        D = [sb([P, maxg, 30, K]) for _ in range(2)]
        SQ = [sb([P, maxg, 32, K]) for _ in range(2)]
        PSB = es.enter_context(nc.sbuf_tensor("psb", [P, 98], F32))
        ps = es.enter_context(nc.psum_tensor("ps", [P, 98], F32))

        sem_names = ["dsemHA", "dsemHB", "dsemTA", "dsemTB", "u_done",
                     "sqrt_done", "ds_done", "clsq", "rm_done", "sqA",
                     "pe_done", "dve_in", "act_in", "pool_in",
                     "cls_dve_g", "cls_pool_g", "obj_done", "rm4_done",
                     "sa_done", "dsemTL", "clsqP", "sqC", "psb_done", "outd"]
        sems = {n: es.enter_context(nc.semaphore(n)) for n in sem_names}
        dsemH = [sems["dsemHA"], sems["dsemHB"]]
        dsemT = [sems["dsemTA"], sems["dsemTB"]]
        u_done = sems["u_done"]
        sqrt_done = sems["sqrt_done"]
        ds_done = sems["ds_done"]
        clsq = sems["clsq"]          # cls squares done, +1 per (chunk, gi)
        rm_done = sems["rm_done"]
        sqA = sems["sqA"]            # coor+conf+noobj squares, +1 per chunk
        pe_done = sems["pe_done"]
        dve_in = sems["dve_in"]
        act_in = sems["act_in"]
        pool_in = sems["pool_in"]
        cls_dve_g = sems["cls_dve_g"]    # cls diffs (dve share) per (chunk,gi)
        cls_pool_g = sems["cls_pool_g"]  # cls diffs (pool share) per (chunk,gi)
        obj_done = sems["obj_done"]
        rm4_done = sems["rm4_done"]
        sa_done = sems["sa_done"]
        dsemTL = sems["dsemTL"]
        clsqP = sems["clsqP"]
        sqC = sems["sqC"]
        psb_done = sems["psb_done"]
        outd = sems["outd"]

        block = es.enter_context(nc.Block())

        offs = [0]
        for g in chunks:
            offs.append(offs[-1] + g * P)

        headv = {}
        tailv = {}
        _hv = [0, 0]
        _tv = [0, 0]
        for i in range(nchunk):
            s = i % 2
            _hv[s] += 32
            _tv[s] += 32
            headv[i] = _hv[s]
            tailv[i] = _tv[s]
        # cumulative (chunk, gi) counter base: number of g-units before chunk i
        gbase = [sum(chunks[:i]) for i in range(nchunk)]

        @block.sync
        def _(sync):
            def head(i):
                g = chunks[i]
                s = i % 2
                if i >= 2:
                    # xh[s] readers of chunk i-2: dve dconf (<= u_done),
                    # act sqno (act_in), pool obj (obj_done)
                    sync.wait_ge(u_done, i - 1)
                    sync.wait_ge(act_in, i - 1)
                    sync.wait_ge(obj_done, i - 1)
                rows = slice(offs[i], offs[i + 1])
                sync.dma_start(
                    out=xh[s][:, 0:g].rearrange("p g c k -> p g (c k)"),
                    in_=x[rows, 0:HX * K].rearrange("(g p) d -> p g d", p=P),
                ).then_inc(dsemH[s], 32)

            def tail(i):
                g = chunks[i]
                s = i % 2
                if i >= 2:
                    # xtl[s] readers of chunk i-2: dve + pool cls diffs
                    sync.wait_ge(cls_dve_g, gbase[i - 1])
                    if clsp > 0:
                        sync.wait_ge(cls_pool_g, gbase[i - 1])
                rows = slice(offs[i], offs[i + 1])
                if i == ilast:
                    for gi in range(g):
                        rg = slice(offs[i] + gi * P, offs[i] + (gi + 1) * P)
                        sync.dma_start(
                            out=xtl[s][:, gi].rearrange("p c k -> p (c k)"),
                            in_=x[rg, HX * K:].rearrange(
                                "(g p) d -> p (g d)", p=P),
                        ).then_inc(dsemTL, 16)
                else:
                    sync.dma_start(
                        out=xtl[s][:, 0:g].rearrange("p g c k -> p g (c k)"),
                        in_=x[rows, HX * K:].rearrange("(g p) d -> p g d", p=P),
                    ).then_inc(dsemT[s], 32)

            for i in range(nchunk):
                head(i)
                tail(i)
            sync.wait_ge(outd, 16)

        @block.gpsimd
        def _(gp):
            for i, g in enumerate(chunks):
                s = i % 2
                if i >= 2:
                    gp.wait_ge(pe_done, i - 1)
                gp.wait_ge(dsemH[s], headv[i])
                gp.tensor_scalar(WT[s][:, 0:g, 8:10, :],
                                 xh[s][:, 0:g, 18:19, :].broadcast_to(
                                     [P, g, 2, K]),
                                 1.0, None,
                                 Alu.is_equal).then_inc(obj_done, 1)
                gp.drain()
                gp.tensor_scalar(WT[s][:, 0:g, 6:8, :], WT[s][:, 0:g, 8:10, :],
                                 -0.5, 0.5, Alu.mult,
                                 Alu.add).then_inc(rm4_done, 1)
                gp.drain().then_inc(pool_in, 1)

                def pool_cls(j):
                    sj = j % 2
                    if clsp == 0:
                        return
                    if j >= 2:
                        gp.wait_ge(sqA, j - 1)
                        gp.wait_ge(sqC, j - 1)
                        gp.wait_ge(clsq, gbase[j - 1])
                        gp.wait_ge(clsqP, gbase[j - 1])
                    for gi in range(chunks[j]):
                        if j == ilast:
                            gp.wait_ge(dsemTL, 16 * (gi + 1))
                        elif gi == 0:
                            gp.wait_ge(dsemT[sj], tailv[j])
                        gp.tensor_tensor(
                            D[sj][:, gi, 10 + clsd:30, :],
                            xtl[sj][:, gi, clsd:20, :],
                            xtl[sj][:, gi, 20 + clsd:40, :], Alu.subtract,
                        ).then_inc(cls_pool_g, 1)
                    gp.drain()

                pool_cls(i)
                if i == ilast:
                    continue  # last-chunk rm built on DVE
                gp.wait_ge(u_done, i + 1)
                gp.tensor_scalar(O5[s][:, 0:g], WT[s][:, 0:g, 8:10, :],
                                 5.0, None, Alu.mult)
                gp.drain()
                gp.tensor_tensor(WT[s][:, 0:g, 0:2, :], O5[s][:, 0:g],
                                 Ut[s][:, 0:g].broadcast_to([P, g, 2, K]),
                                 Alu.mult)
                gp.tensor_scalar(TQ[s][:, 0:g, 0:1, :], Ut[s][:, 0:g],
                                 0.5, 0.5, Alu.mult, Alu.add)
                gp.tensor_scalar(TQ[s][:, 0:g, 1:2, :], Ut[s][:, 0:g],
                                 -0.5, 1.0, Alu.mult, Alu.add)
                gp.drain()
                gp.tensor_tensor(WT[s][:, 0:g, 2:4, :], O5[s][:, 0:g],
                                 WT[s][:, 0:g, 0:2, :], Alu.subtract)
                gp.tensor_tensor(
                    WT[s][:, 0:g, 4:6, :], TQ[s][:, 0:g],
                    WT[s][:, 0:g, 8:10, :], Alu.mult,
                ).then_inc(rm_done, 1)
                gp.drain()

        @block.scalar
        def _(act):
            for i, g in enumerate(chunks):
                s = i % 2
                if i >= 2:
                    act.wait_ge(pe_done, i - 1)   # SQ reuse
                    act.wait_ge(ds_done, i - 1)   # SQP/SQL reuse
                act.wait_ge(dsemH[s], headv[i])
                act.activation(SQP[s][:, 0:g], xh[s][:, 0:g, 4:8, :],
                               Act.Sqrt)
                act.activation(SQL[s][:, 0:g], xh[s][:, 0:g, 14:18, :],
                               Act.Sqrt).then_inc(sqrt_done, 1)
                act.activation(SQ[s][:, 0:g, 30:32, :],
                               xh[s][:, 0:g, 8:10, :],
                               Act.Square).then_inc(act_in, 1)
                for gi in range(g):
                    act.wait_ge(cls_dve_g, gbase[i] + gi + 1)
                    act.activation(SQ[s][:, gi, 10:10 + clsd, :],
                                   D[s][:, gi, 10:10 + clsd, :],
                                   Act.Square).then_inc(clsq, 1)
                act.wait_ge(ds_done, i + 1)
                act.activation(SQ[s][:, 0:g, 0:8, :],
                               D[s][:, 0:g, 0:8, :],
                               Act.Square).then_inc(sqC, 1)
                for gi in range(g):
                    act.wait_ge(cls_pool_g, gbase[i] + gi + 1)
                    act.activation(SQ[s][:, gi, 10 + clsd:30, :],
                                   D[s][:, gi, 10 + clsd:30, :],
                                   Act.Square).then_inc(clsqP, 1)
                act.wait_ge(u_done, i + 1)
                act.activation(SQ[s][:, 0:g, 8:10, :],
                               D[s][:, 0:g, 8:10, :],
                               Act.Square).then_inc(sqA, 1)
            act.wait_ge(pe_done, nchunk)
            act.activation(PSB[0:98, :], ps[0:98, :],
                           Act.Copy).then_inc(psb_done, 1)
            act.dma_start(out=out2[:], in_=PSB[:]).then_inc(outd, 16)

        @block.vector
        def _(v):
            tt = v.tensor_tensor
            ts = v.tensor_scalar

            def cls_fill(j, gi):
                """emit chunk j's cls diff for unit gi (waits its tail dma)"""
                sj = j % 2
                if j == ilast:
                    v.wait_ge(dsemTL, 16 * (gi + 1))
                elif gi == 0:
                    v.wait_ge(dsemT[sj], tailv[j])
                tt(D[sj][:, gi, 10:10 + clsd, :],
                   xtl[sj][:, gi, 0:clsd, :],
                   xtl[sj][:, gi, 20:20 + clsd, :],
                   Alu.subtract).then_inc(cls_dve_g, 1)

            for i, g in enumerate(chunks):
                s = i % 2
                last = (i == ilast)
                p = xh[s]
                # cls fills ride in this chunk's own chain drain slots
                fills = [] if last else [(i, gi) for gi in range(g)]
                if i >= 2:
                    v.wait_ge(sqA, i - 1)      # D reuse (act read done)
                    v.wait_ge(sqC, i - 1)
                    v.wait_ge(clsq, gbase[i - 1])  # prev-parity cls read
                    v.wait_ge(clsqP, gbase[i - 1])
                    v.wait_ge(rm_done, i - 1)  # Ut reuse
                    if cfg["sq_coor"] == "dve":
                        v.wait_ge(pe_done, i - 1)  # SQ reuse
                if last:
                    # cls diffs first: tail DMA streams while the previous
                    # chunk still computes; Act/PE stream cls early
                    for gi in range(g):
                        cls_fill(i, gi)
                v.wait_ge(dsemH[s], headv[i])
                gxy2 = (p[:, 0:g, 10:12, :].unsqueeze(2)
                        .broadcast_to([P, g, 2, 2, K]))
                gwh2 = (p[:, 0:g, 14:16, :].unsqueeze(2)
                        .broadcast_to([P, g, 2, 2, K]))
                pxy_v = p[:, 0:g, 0:4, :].rearrange(
                    "p g (b c) k -> p g b c k", b=2)
                pwh_v = p[:, 0:g, 4:8, :].rearrange(
                    "p g (b c) k -> p g b c k", b=2)
                # W1
                tt(XT[s][:, 0:g].rearrange("p g (b c) k -> p g b c k", b=2),
                   pxy_v, gxy2, Alu.subtract)
                tt(D[s][:, 0:g, 2:4, :], p[:, 0:g, 2:4, :],
                   p[:, 0:g, 12:14, :], Alu.subtract)
                tt(SM[s][:, 0:g, 0:4, :].rearrange(
                    "p g (b c) k -> p g b c k", b=2),
                   pwh_v, gwh2, Alu.add)
                tt(SM[s][:, 0:g, 4:8, :].rearrange(
                    "p g (b c) k -> p g b c k", b=2),
                   pwh_v, gwh2, Alu.min)
                tt(AR[s][:, 0:g, 0:2, :],
                   pwh_v[:, :, :, 0, :], pwh_v[:, :, :, 1, :], Alu.mult)
                tt(AR[s][:, 0:g, 2:3, :], p[:, 0:g, 14:15, :],
                   p[:, 0:g, 15:16, :], Alu.mult)
                v.drain()
                # W1b
                v.tensor_copy(D[s][:, 0:g, 0:2, :], XT[s][:, 0:g, 0:2, :])
                ts(AD[s][:, 0:g].bitcast(U16), XT[s][:, 0:g].bitcast(U16),
                   0x7FFF, None, Alu.bitwise_and)
                ts(SM2[s][:, 0:g, 0:4, :], SM[s][:, 0:g, 0:4, :],
                   3.5, None, Alu.mult)
                ts(SM2[s][:, 0:g, 4:8, :], SM[s][:, 0:g, 4:8, :],
                   7.0, None, Alu.mult)
                tt(SA[s][:, 0:g], AR[s][:, 0:g, 0:2, :],
                   AR[s][:, 0:g, 2:3, :].broadcast_to([P, g, 2, K]),
                   Alu.add)
                v.drain()
                # W2 chain; prev chunk's cls diffs + ds fill the drain slots
                tt(TA[s][:, 0:g], SM2[s][:, 0:g, 0:4, :], AD[s][:, 0:g],
                   Alu.subtract)
                v.wait_ge(sqrt_done, i + 1)
                tt(D[s][:, 0:g, 4:8, :], SQP[s][:, 0:g], SQL[s][:, 0:g],
                   Alu.subtract).then_inc(ds_done, 1)
                v.drain()
                tt(OV[s][:, 0:g], SM2[s][:, 0:g, 4:8, :], TA[s][:, 0:g],
                   Alu.min)
                if fills:
                    cls_fill(*fills.pop(0))
                v.drain()
                ts(CLt[s][:, 0:g], OV[s][:, 0:g], 0.0, None, Alu.max)
                if fills:
                    cls_fill(*fills.pop(0))
                v.drain()
                clv = CLt[s][:, 0:g].rearrange("p g (b c) k -> p g b c k",
                                               b=2)
                tt(INT[s][:, 0:g], clv[:, :, :, 0, :], clv[:, :, :, 1, :],
                   Alu.mult)
                if fills:
                    cls_fill(*fills.pop(0))
                v.drain()
                ts(SA49[s][:, 0:g], SA[s][:, 0:g], 49.0, None, Alu.mult)
                if fills:
                    cls_fill(*fills.pop(0))
                v.drain()
                tt(U49[s][:, 0:g], SA49[s][:, 0:g], INT[s][:, 0:g],
                   Alu.subtract)
                v.drain()
                with nc.allow_low_precision(reason="bf16 iou ok"):
                    v.reciprocal(RCPt[s][:, 0:g], U49[s][:, 0:g])
                v.drain()
                tt(IOU[s][:, 0:g], INT[s][:, 0:g], RCPt[s][:, 0:g], Alu.mult)
                v.drain()
                tt(Ut[s][:, 0:g], IOU[s][:, 0:g, 0:1, :],
                   IOU[s][:, 0:g, 1:2, :], Alu.is_ge)
                tt(D[s][:, 0:g, 8:10, :], p[:, 0:g, 8:10, :], IOU[s][:, 0:g],
                   Alu.subtract)
                v.drain().then_inc(u_done, 1)
                if cfg["sq_coor"] == "dve":
                    tt(SQ[s][:, 0:g, 0:8, :], D[s][:, 0:g, 0:8, :],
                       D[s][:, 0:g, 0:8, :], Alu.mult).then_inc(sqA, 1)
                if not last:
                    while fills:
                        cls_fill(*fills.pop(0))
                    v.drain().then_inc(dve_in, 1)
                else:
                    # last chunk: rm build on DVE (shortest tail)
                    v.wait_ge(obj_done, i + 1)
                    ts(TQ[s][:, 0:g, 0:1, :], Ut[s][:, 0:g],
                       0.5, 0.5, Alu.mult, Alu.add)
                    ts(TQ[s][:, 0:g, 1:2, :], Ut[s][:, 0:g],
                       -0.5, 1.0, Alu.mult, Alu.add)
                    ts(O5[s][:, 0:g], WT[s][:, 0:g, 8:10, :],
                       5.0, None, Alu.mult)
                    v.drain().then_inc(dve_in, 1)
                    tt(WT[s][:, 0:g, 0:2, :], O5[s][:, 0:g],
                       Ut[s][:, 0:g].broadcast_to([P, g, 2, K]), Alu.mult)
                    tt(WT[s][:, 0:g, 4:6, :], TQ[s][:, 0:g],
                       WT[s][:, 0:g, 8:10, :], Alu.mult)
                    v.drain()
                    tt(WT[s][:, 0:g, 2:4, :], O5[s][:, 0:g],
                       WT[s][:, 0:g, 0:2, :],
                       Alu.subtract).then_inc(rm_done, 1)
                    v.drain()
                    while fills:
                        cls_fill(*fills.pop(0))
                    v.drain()

        @block.tensor
        def _(pe):
            first = True
            for i, g in enumerate(chunks):
                s = i % 2

                def mm(c, w, gi, last=False):
                    nonlocal first
                    r = pe.matmul(
                        ps[0:98, :],
                        WT[s][:, gi, w:w + 2, :].rearrange(
                            "p c k -> p (c k)"),
                        SQ[s][:, gi, c:c + 2, :].rearrange(
                            "p c k -> p (c k)"),
                        start=first, stop=last, skip_group_check=True)
                    first = False
                    return r

                # wave B: cls blocks; dve-share pairs stream first
                for gi in range(g):
                    pe.wait_ge(clsq, gbase[i] + gi + 1)
                    if gi == 0:
                        pe.wait_ge(rm4_done, i + 1)
                    for c in range(10, 10 + clsd, 2):
                        mm(c, 8, gi)
                for gi in range(g):
                    pe.wait_ge(clsqP, gbase[i] + gi + 1)
                    for c in range(10 + clsd, 30, 2):
                        mm(c, 8, gi)
                # wave A: coor (sqC-gated) then conf+noobj (sqA-gated)
                pe.wait_ge(sqC, i + 1)
                pe.wait_ge(rm_done, i + 1)
                for gi in range(g):
                    mm(0, 0, gi)
                    mm(2, 2, gi)
                    mm(4, 0, gi)
                    mm(6, 2, gi)
                pe.wait_ge(sqA, i + 1)
                for gi in range(g):
                    mm(8, 4, gi)
                    lastmm = (i == nchunk - 1 and gi == g - 1)
                    r = mm(30, 6, gi, last=lastmm)
                    if gi == g - 1:
                        r.then_inc(pe_done, 1)

    return nc


_NC_CACHE = {}


def _get_nc():
    if "nc" not in _NC_CACHE:
        _NC_CACHE["nc"] = build_nc()
    return _NC_CACHE["nc"]


def _to_bf16_repack(pred, labels):
    import ml_dtypes

    bf = ml_dtypes.bfloat16
    p = np.ascontiguousarray(pred, dtype=np.float32).reshape(B_TOTAL, 30, K)
    l = np.ascontiguousarray(labels, dtype=np.float32).reshape(B_TOTAL, 30, K)
    pb = p.astype(bf)
    lb = l.astype(bf)
    # obj channel: keep the ==1.0 test exact under rounding
    l4 = l[:, 4, :]
    lb4 = lb[:, 4, :]
    bad = (l4 != np.float32(1.0)) & (lb4.astype(np.float32) == np.float32(1.0))
    if bad.any():
        lb4[bad] = bf(0.99609375)
        lb[:, 4, :] = lb4
    xall = np.concatenate(
        [pb[:, PP_IDX[:PH], :], lb[:, LL_IDX[:LH], :],
         pb[:, PP_IDX[PH:], :], lb[:, LL_IDX[LH:], :]], axis=1)
    return np.ascontiguousarray(xall).reshape(B_TOTAL, ROWX)


def run_device(pred, labels, trace=False):
    from concourse.bass_utils import run_bass_kernel_spmd

    nc = _get_nc()
    xrp = _to_bf16_repack(pred, labels)
    in_maps = []
    for c in range(NCORES):
        rows = slice(c * B_CORE, (c + 1) * B_CORE)
        in_maps.append({"x": xrp[rows]})
    res = run_bass_kernel_spmd(nc, in_maps, list(range(NCORES)), trace=trace)
    total = 0.0
    for c in range(NCORES):
        m = res.results[c]["out2"].astype(np.float64)
        total += float(np.trace(m[0:98, 0:98]))
    loss = np.float32(total / B_TOTAL)
    return loss, res


def kernel(pred, labels):
    loss, _ = run_device(pred, labels, trace=False)
    return np.array(loss, dtype=np.float32)


if __name__ == "__main__":
    rng = np.random.default_rng(0)
    p = rng.random((B_TOTAL, 30, 7, 7), dtype=np.float32)
    l = rng.random((B_TOTAL, 30, 7, 7), dtype=np.float32)
    l[:, 4] = (rng.random((B_TOTAL, 7, 7)) < 0.3).astype(np.float32)
    print(kernel(p, l))


# revision 6
# speedup vs baseline: 1.0398x; 1.0068x over previous
"""YOLOv1 loss kernel v2 for Trainium2, 8-core data-parallel, bf16.

Layout per core (1024 rows, chunks of g*128 rows, tiles [128, g, ch, 49]):
  pred  PP (30 ch): [x1,y1,x2,y2, w1,h1,w2,h2, c1,c2, cls*20]
  label LL (29 ch): [gx,gy, x5,y5, gw,gh, w7,h7, obj, cls*20]

Math: iou via 7x-scaled overlap ov = relu(min(7*min(w,wg), 3.5*(w+wg)-|dc|)),
ints = ovx*ovy, u49 = 49*(areas)-ints, iou = ints/u49.
All loss terms become sum_cells w_ch * d_ch^2 with per-cell weights:
  coor (D 0:8 = [c1x,c1y,c2x,c2y, s1w,s1h,s2w,s2h]): w = 5*obj*u / 5*obj*(1-u)
  conf (D 8:10 = dconf): w = obj*(.5+.5u) / obj*(1-.5u)
  cls  (D 10:30): w = obj
  noobj (pred conf raw, squared into SQ 30:32): w = .5*(1-obj)
Squares SQ = D^2 (Act; DVE self-mult on the last chunk); the weighted
accumulation runs on the idle PE: per 2-channel block,
matmul(psum[0:98,0:98], lhsT=weight-broadcast, rhs=SQ-block) accumulated over
all blocks/chunks; diag(psum)[i] = sum_p w[p,i]*sq[p,i]. Host sums the psum
diagonal (f64) -> loss.

Engines: DVE = diffs/iou pipeline; Pool = obj/weight builds/cls share;
Act = sqrt(5*w), squares; PE = weighted accumulation; SP = DMA.
cls diffs/squares pipeline per g-unit so PE streams behind Act.
"""

import sys

import numpy as np

for _p in ("/opt/trn_rl_repo", "/root/.axon_site/_ro/trn_rl_repo"):
    if _p not in sys.path:
        sys.path.insert(0, _p)

import concourse.bass as bass
import concourse.mybir as mybir

F32 = mybir.dt.float32
BF16 = mybir.dt.bfloat16
U16 = mybir.dt.uint16
Alu = mybir.AluOpType
Act = mybir.ActivationFunctionType

B_TOTAL = 8192
NCORES = 8
B_CORE = B_TOTAL // NCORES  # 1024
P = 128
K = 49
CP = 30
CL = 29
ROWP = CP * K
ROWL = CL * K

PP_IDX = [0, 1, 5, 6, 2, 3, 7, 8, 4, 9] + list(range(10, 30))
LL_IDX = [0, 1, 5, 6, 2, 3, 7, 8, 4] + list(range(10, 30))

PH = 10
LH = 9
CX = CP + CL          # 59 combined channels per row
HX = PH + LH          # 19 head channels
ROWX = CX * K

CFG = dict(
    chunks=(2, 3, 3),
    cls_pool=10,      # cls diff channels done on Pool (rest on DVE)
    sq_coor="act",    # coor squares on act or dve
    act_split=False,  # conf square separate from coor square on Act
)


def build_nc(cfg=None):
    cfg = dict(CFG, **(cfg or {}))
    chunks = cfg["chunks"]
    clsp = cfg["cls_pool"]
    clsd = 20 - clsp
    assert sum(chunks) * P == B_CORE
    nchunk = len(chunks)
    ilast = nchunk - 1
    maxg = max(chunks)
    nc = bass.Bass()
    x = nc.declare_dram_parameter("x", [B_CORE, ROWX], BF16, isOutput=False)
    out2 = nc.declare_dram_parameter("out2", [P, 98], F32, isOutput=True)

    from contextlib import ExitStack

    _ctr = [0]
    es = ExitStack()

    def sb(shape, dt=BF16):
        _ctr[0] += 1
        return es.enter_context(nc.sbuf_tensor(f"t{_ctr[0]}", shape, dt))

    with es:
        # combined tiles: xh = head (pred 0:10 | label 0:9),
        # xtl = tail (pred cls 20 | label cls 20)
        xh = [sb([P, maxg, HX, K]) for _ in range(2)]
        xtl = [sb([P, maxg, 40, K]) for _ in range(2)]
        XT = [sb([P, maxg, 4, K]) for _ in range(2)]
        AD = [sb([P, maxg, 4, K]) for _ in range(2)]
        SM = [sb([P, maxg, 8, K]) for _ in range(2)]
        SM2 = [sb([P, maxg, 8, K]) for _ in range(2)]
        TA = [sb([P, maxg, 4, K]) for _ in range(2)]
        OV = [sb([P, maxg, 4, K]) for _ in range(2)]
        CLt = [sb([P, maxg, 4, K]) for _ in range(2)]
        AR = [sb([P, maxg, 3, K]) for _ in range(2)]
        SA = [sb([P, maxg, 2, K]) for _ in range(2)]
        SA49 = [sb([P, maxg, 2, K]) for _ in range(2)]
        INT = [sb([P, maxg, 2, K]) for _ in range(2)]
        U49 = [sb([P, maxg, 2, K]) for _ in range(2)]
        RCPt = [sb([P, maxg, 2, K]) for _ in range(2)]
        IOU = [sb([P, maxg, 2, K]) for _ in range(2)]
        Ut = [sb([P, maxg, 1, K]) for _ in range(2)]
        O5 = [sb([P, maxg, 2, K]) for _ in range(2)]
        TQ = [sb([P, maxg, 2, K]) for _ in range(2)]
        # weight tile: [rm0,rm0 | rm1,rm1 | rm2,rm3 | rm4,rm4 | obj,obj]
        WT = [sb([P, maxg, 10, K]) for _ in range(2)]
        SQP = [sb([P, maxg, 4, K]) for _ in range(2)]
        SQL = [sb([P, maxg, 4, K]) for _ in range(2)]
        D = [sb([P, maxg, 30, K]) for _ in range(2)]
        SQ = [sb([P, maxg, 32, K]) for _ in range(2)]
        PSB = es.enter_context(nc.sbuf_tensor("psb", [P, 98], F32))
        ps = es.enter_context(nc.psum_tensor("ps", [P, 98], F32))

        sem_names = ["dsemHA", "dsemHB", "dsemTA", "dsemTB", "u_done",
                     "sqrt_done", "ds_done", "clsq", "rm_done", "sqA",
                     "pe_done", "dve_in", "act_in", "pool_in",
                     "cls_dve_g", "cls_pool_g", "obj_done", "rm4_done",
                     "sa_done", "dsemTL", "clsqP", "sqC", "rm01", "psb_done", "outd"]
        sems = {n: es.enter_context(nc.semaphore(n)) for n in sem_names}
        dsemH = [sems["dsemHA"], sems["dsemHB"]]
        dsemT = [sems["dsemTA"], sems["dsemTB"]]
        u_done = sems["u_done"]
        sqrt_done = sems["sqrt_done"]
        ds_done = sems["ds_done"]
        clsq = sems["clsq"]          # cls squares done, +1 per (chunk, gi)
        rm_done = sems["rm_done"]
        sqA = sems["sqA"]            # coor+conf+noobj squares, +1 per chunk
        pe_done = sems["pe_done"]
        dve_in = sems["dve_in"]
        act_in = sems["act_in"]
        pool_in = sems["pool_in"]
        cls_dve_g = sems["cls_dve_g"]    # cls diffs (dve share) per (chunk,gi)
        cls_pool_g = sems["cls_pool_g"]  # cls diffs (pool share) per (chunk,gi)
        obj_done = sems["obj_done"]
        rm4_done = sems["rm4_done"]
        sa_done = sems["sa_done"]
        dsemTL = sems["dsemTL"]
        clsqP = sems["clsqP"]
        sqC = sems["sqC"]
        rm01 = sems["rm01"]
        psb_done = sems["psb_done"]
        outd = sems["outd"]

        block = es.enter_context(nc.Block())

        offs = [0]
        for g in chunks:
            offs.append(offs[-1] + g * P)

        headv = {}
        tailv = {}
        _hv = [0, 0]
        _tv = [0, 0]
        for i in range(nchunk):
            s = i % 2
            _hv[s] += 32
            _tv[s] += 32
            headv[i] = _hv[s]
            tailv[i] = _tv[s]
        # cumulative (chunk, gi) counter base: number of g-units before chunk i
        gbase = [sum(chunks[:i]) for i in range(nchunk)]

        @block.sync
        def _(sync):
            def head(i):
                g = chunks[i]
                s = i % 2
                if i >= 2:
                    # xh[s] readers of chunk i-2: dve dconf (<= u_done),
                    # act sqno (act_in), pool obj (obj_done)
                    sync.wait_ge(u_done, i - 1)
                    sync.wait_ge(act_in, i - 1)
                    sync.wait_ge(obj_done, i - 1)
                rows = slice(offs[i], offs[i + 1])
                sync.dma_start(
                    out=xh[s][:, 0:g].rearrange("p g c k -> p g (c k)"),
                    in_=x[rows, 0:HX * K].rearrange("(g p) d -> p g d", p=P),
                ).then_inc(dsemH[s], 32)

            def tail(i):
                g = chunks[i]
                s = i % 2
                if i >= 2:
                    # xtl[s] readers of chunk i-2: dve + pool cls diffs
                    sync.wait_ge(cls_dve_g, gbase[i - 1])
                    if clsp > 0:
                        sync.wait_ge(cls_pool_g, gbase[i - 1])
                rows = slice(offs[i], offs[i + 1])
                if i == ilast:
                    for gi in range(g):
                        rg = slice(offs[i] + gi * P, offs[i] + (gi + 1) * P)
                        sync.dma_start(
                            out=xtl[s][:, gi].rearrange("p c k -> p (c k)"),
                            in_=x[rg, HX * K:].rearrange(
                                "(g p) d -> p (g d)", p=P),
                        ).then_inc(dsemTL, 16)
                else:
                    sync.dma_start(
                        out=xtl[s][:, 0:g].rearrange("p g c k -> p g (c k)"),
                        in_=x[rows, HX * K:].rearrange("(g p) d -> p g d", p=P),
                    ).then_inc(dsemT[s], 32)

            for i in range(nchunk):
                head(i)
                tail(i)
            sync.wait_ge(outd, 16)

        @block.gpsimd
        def _(gp):
            for i, g in enumerate(chunks):
                s = i % 2
                if i >= 2:
                    gp.wait_ge(pe_done, i - 1)
                gp.wait_ge(dsemH[s], headv[i])
                gp.tensor_scalar(WT[s][:, 0:g, 8:10, :],
                                 xh[s][:, 0:g, 18:19, :].broadcast_to(
                                     [P, g, 2, K]),
                                 1.0, None,
                                 Alu.is_equal).then_inc(obj_done, 1)
                gp.drain()
                gp.tensor_scalar(WT[s][:, 0:g, 6:8, :], WT[s][:, 0:g, 8:10, :],
                                 -0.5, 0.5, Alu.mult,
                                 Alu.add).then_inc(rm4_done, 1)
                gp.drain().then_inc(pool_in, 1)

                def pool_cls(j):
                    sj = j % 2
                    if clsp == 0:
                        return
                    if j >= 2:
                        gp.wait_ge(sqA, j - 1)
                        gp.wait_ge(sqC, j - 1)
                        gp.wait_ge(clsq, gbase[j - 1])
                        gp.wait_ge(clsqP, gbase[j - 1])
                    for gi in range(chunks[j]):
                        if j == ilast:
                            gp.wait_ge(dsemTL, 16 * (gi + 1))
                        elif gi == 0:
                            gp.wait_ge(dsemT[sj], tailv[j])
                        gp.tensor_tensor(
                            D[sj][:, gi, 10 + clsd:30, :],
                            xtl[sj][:, gi, clsd:20, :],
                            xtl[sj][:, gi, 20 + clsd:40, :], Alu.subtract,
                        ).then_inc(cls_pool_g, 1)
                    gp.drain()

                pool_cls(i)
                if i == ilast:
                    continue  # last-chunk rm built on DVE
                gp.wait_ge(u_done, i + 1)
                gp.tensor_scalar(O5[s][:, 0:g], WT[s][:, 0:g, 8:10, :],
                                 5.0, None, Alu.mult)
                gp.drain()
                gp.tensor_tensor(WT[s][:, 0:g, 0:2, :], O5[s][:, 0:g],
                                 Ut[s][:, 0:g].broadcast_to([P, g, 2, K]),
                                 Alu.mult)
                gp.tensor_scalar(TQ[s][:, 0:g, 0:1, :], Ut[s][:, 0:g],
                                 0.5, 0.5, Alu.mult, Alu.add)
                gp.tensor_scalar(TQ[s][:, 0:g, 1:2, :], Ut[s][:, 0:g],
                                 -0.5, 1.0, Alu.mult, Alu.add)
                gp.drain()
                gp.tensor_tensor(WT[s][:, 0:g, 2:4, :], O5[s][:, 0:g],
                                 WT[s][:, 0:g, 0:2, :],
                                 Alu.subtract).then_inc(rm01, 1)
                gp.tensor_tensor(
                    WT[s][:, 0:g, 4:6, :], TQ[s][:, 0:g],
                    WT[s][:, 0:g, 8:10, :], Alu.mult,
                ).then_inc(rm_done, 1)
                gp.drain()

        @block.scalar
        def _(act):
            for i, g in enumerate(chunks):
                s = i % 2
                if i >= 2:
                    act.wait_ge(pe_done, i - 1)   # SQ reuse
                    act.wait_ge(ds_done, i - 1)   # SQP/SQL reuse
                act.wait_ge(dsemH[s], headv[i])
                act.activation(SQP[s][:, 0:g], xh[s][:, 0:g, 4:8, :],
                               Act.Sqrt)
                act.activation(SQL[s][:, 0:g], xh[s][:, 0:g, 14:18, :],
                               Act.Sqrt).then_inc(sqrt_done, 1)
                act.activation(SQ[s][:, 0:g, 30:32, :],
                               xh[s][:, 0:g, 8:10, :],
                               Act.Square).then_inc(act_in, 1)
                for gi in range(g):
                    act.wait_ge(cls_dve_g, gbase[i] + gi + 1)
                    act.activation(SQ[s][:, gi, 10:10 + clsd, :],
                                   D[s][:, gi, 10:10 + clsd, :],
                                   Act.Square).then_inc(clsq, 1)
                act.wait_ge(ds_done, i + 1)
                act.activation(SQ[s][:, 0:g, 0:8, :],
                               D[s][:, 0:g, 0:8, :],
                               Act.Square).then_inc(sqC, 1)
                for gi in range(g):
                    act.wait_ge(cls_pool_g, gbase[i] + gi + 1)
                    act.activation(SQ[s][:, gi, 10 + clsd:30, :],
                                   D[s][:, gi, 10 + clsd:30, :],
                                   Act.Square).then_inc(clsqP, 1)
                act.wait_ge(u_done, i + 1)
                act.activation(SQ[s][:, 0:g, 8:10, :],
                               D[s][:, 0:g, 8:10, :],
                               Act.Square).then_inc(sqA, 1)
            act.wait_ge(pe_done, nchunk)
            act.activation(PSB[0:98, :], ps[0:98, :],
                           Act.Copy).then_inc(psb_done, 1)
            act.dma_start(out=out2[:], in_=PSB[:]).then_inc(outd, 16)

        @block.vector
        def _(v):
            tt = v.tensor_tensor
            ts = v.tensor_scalar

            def cls_fill(j, gi):
                """emit chunk j's cls diff for unit gi (waits its tail dma)"""
                sj = j % 2
                if j == ilast:
                    v.wait_ge(dsemTL, 16 * (gi + 1))
                elif gi == 0:
                    v.wait_ge(dsemT[sj], tailv[j])
                tt(D[sj][:, gi, 10:10 + clsd, :],
                   xtl[sj][:, gi, 0:clsd, :],
                   xtl[sj][:, gi, 20:20 + clsd, :],
                   Alu.subtract).then_inc(cls_dve_g, 1)

            for i, g in enumerate(chunks):
                s = i % 2
                last = (i == ilast)
                p = xh[s]
                # cls fills ride in this chunk's own chain drain slots
                fills = [] if last else [(i, gi) for gi in range(g)]
                if i >= 2:
                    v.wait_ge(sqA, i - 1)      # D reuse (act read done)
                    v.wait_ge(sqC, i - 1)
                    v.wait_ge(clsq, gbase[i - 1])  # prev-parity cls read
                    v.wait_ge(clsqP, gbase[i - 1])
                    v.wait_ge(rm_done, i - 1)  # Ut reuse
                    if cfg["sq_coor"] == "dve":
                        v.wait_ge(pe_done, i - 1)  # SQ reuse
                if last:
                    # cls diffs first: tail DMA streams while the previous
                    # chunk still computes; Act/PE stream cls early
                    for gi in range(g):
                        cls_fill(i, gi)
                v.wait_ge(dsemH[s], headv[i])
                gxy2 = (p[:, 0:g, 10:12, :].unsqueeze(2)
                        .broadcast_to([P, g, 2, 2, K]))
                gwh2 = (p[:, 0:g, 14:16, :].unsqueeze(2)
                        .broadcast_to([P, g, 2, 2, K]))
                pxy_v = p[:, 0:g, 0:4, :].rearrange(
                    "p g (b c) k -> p g b c k", b=2)
                pwh_v = p[:, 0:g, 4:8, :].rearrange(
                    "p g (b c) k -> p g b c k", b=2)
                # W1
                tt(XT[s][:, 0:g].rearrange("p g (b c) k -> p g b c k", b=2),
                   pxy_v, gxy2, Alu.subtract)
                tt(D[s][:, 0:g, 2:4, :], p[:, 0:g, 2:4, :],
                   p[:, 0:g, 12:14, :], Alu.subtract)
                tt(SM[s][:, 0:g, 0:4, :].rearrange(
                    "p g (b c) k -> p g b c k", b=2),
                   pwh_v, gwh2, Alu.add)
                tt(SM[s][:, 0:g, 4:8, :].rearrange(
                    "p g (b c) k -> p g b c k", b=2),
                   pwh_v, gwh2, Alu.min)
                tt(AR[s][:, 0:g, 0:2, :],
                   pwh_v[:, :, :, 0, :], pwh_v[:, :, :, 1, :], Alu.mult)
                tt(AR[s][:, 0:g, 2:3, :], p[:, 0:g, 14:15, :],
                   p[:, 0:g, 15:16, :], Alu.mult)
                v.drain()
                # W1b
                v.tensor_copy(D[s][:, 0:g, 0:2, :], XT[s][:, 0:g, 0:2, :])
                ts(AD[s][:, 0:g].bitcast(U16), XT[s][:, 0:g].bitcast(U16),
                   0x7FFF, None, Alu.bitwise_and)
                ts(SM2[s][:, 0:g, 0:4, :], SM[s][:, 0:g, 0:4, :],
                   3.5, None, Alu.mult)
                ts(SM2[s][:, 0:g, 4:8, :], SM[s][:, 0:g, 4:8, :],
                   7.0, None, Alu.mult)
                tt(SA[s][:, 0:g], AR[s][:, 0:g, 0:2, :],
                   AR[s][:, 0:g, 2:3, :].broadcast_to([P, g, 2, K]),
                   Alu.add)
                v.drain()
                # W2 chain; prev chunk's cls diffs + ds fill the drain slots
                tt(TA[s][:, 0:g], SM2[s][:, 0:g, 0:4, :], AD[s][:, 0:g],
                   Alu.subtract)
                v.wait_ge(sqrt_done, i + 1)
                tt(D[s][:, 0:g, 4:8, :], SQP[s][:, 0:g], SQL[s][:, 0:g],
                   Alu.subtract).then_inc(ds_done, 1)
                v.drain()
                tt(OV[s][:, 0:g], SM2[s][:, 0:g, 4:8, :], TA[s][:, 0:g],
                   Alu.min)
                if fills:
                    cls_fill(*fills.pop(0))
                v.drain()
                ts(CLt[s][:, 0:g], OV[s][:, 0:g], 0.0, None, Alu.max)
                if fills:
                    cls_fill(*fills.pop(0))
                v.drain()
                clv = CLt[s][:, 0:g].rearrange("p g (b c) k -> p g b c k",
                                               b=2)
                tt(INT[s][:, 0:g], clv[:, :, :, 0, :], clv[:, :, :, 1, :],
                   Alu.mult)
                if fills:
                    cls_fill(*fills.pop(0))
                v.drain()
                ts(SA49[s][:, 0:g], SA[s][:, 0:g], 49.0, None, Alu.mult)
                if fills:
                    cls_fill(*fills.pop(0))
                v.drain()
                tt(U49[s][:, 0:g], SA49[s][:, 0:g], INT[s][:, 0:g],
                   Alu.subtract)
                v.drain()
                with nc.allow_low_precision(reason="bf16 iou ok"):
                    v.reciprocal(RCPt[s][:, 0:g], U49[s][:, 0:g])
                v.drain()
                tt(IOU[s][:, 0:g], INT[s][:, 0:g], RCPt[s][:, 0:g], Alu.mult)
                v.drain()
                tt(Ut[s][:, 0:g], IOU[s][:, 0:g, 0:1, :],
                   IOU[s][:, 0:g, 1:2, :], Alu.is_ge)
                tt(D[s][:, 0:g, 8:10, :], p[:, 0:g, 8:10, :], IOU[s][:, 0:g],
                   Alu.subtract)
                v.drain().then_inc(u_done, 1)
                if cfg["sq_coor"] == "dve":
                    tt(SQ[s][:, 0:g, 0:8, :], D[s][:, 0:g, 0:8, :],
                       D[s][:, 0:g, 0:8, :], Alu.mult).then_inc(sqA, 1)
                if not last:
                    while fills:
                        cls_fill(*fills.pop(0))
                    v.drain().then_inc(dve_in, 1)
                else:
                    # last chunk: rm build on DVE (shortest tail)
                    v.wait_ge(obj_done, i + 1)
                    ts(TQ[s][:, 0:g, 0:1, :], Ut[s][:, 0:g],
                       0.5, 0.5, Alu.mult, Alu.add)
                    ts(TQ[s][:, 0:g, 1:2, :], Ut[s][:, 0:g],
                       -0.5, 1.0, Alu.mult, Alu.add)
                    ts(O5[s][:, 0:g], WT[s][:, 0:g, 8:10, :],
                       5.0, None, Alu.mult)
                    v.drain().then_inc(dve_in, 1)
                    tt(WT[s][:, 0:g, 0:2, :], O5[s][:, 0:g],
                       Ut[s][:, 0:g].broadcast_to([P, g, 2, K]), Alu.mult)
                    v.drain()
                    tt(WT[s][:, 0:g, 2:4, :], O5[s][:, 0:g],
                       WT[s][:, 0:g, 0:2, :],
                       Alu.subtract).then_inc(rm01, 1)
                    tt(WT[s][:, 0:g, 4:6, :], TQ[s][:, 0:g],
                       WT[s][:, 0:g, 8:10, :],
                       Alu.mult).then_inc(rm_done, 1)
                    v.drain()
                    while fills:
                        cls_fill(*fills.pop(0))
                    v.drain()

        @block.tensor
        def _(pe):
            first = True
            for i, g in enumerate(chunks):
                s = i % 2

                def mm(c, w, gi, last=False):
                    nonlocal first
                    r = pe.matmul(
                        ps[0:98, :],
                        WT[s][:, gi, w:w + 2, :].rearrange(
                            "p c k -> p (c k)"),
                        SQ[s][:, gi, c:c + 2, :].rearrange(
                            "p c k -> p (c k)"),
                        start=first, stop=last, skip_group_check=True)
                    first = False
                    return r

                # wave B: cls blocks; dve-share pairs stream first
                for gi in range(g):
                    pe.wait_ge(clsq, gbase[i] + gi + 1)
                    if gi == 0:
                        pe.wait_ge(rm4_done, i + 1)
                    for c in range(10, 10 + clsd, 2):
                        mm(c, 8, gi)
                for gi in range(g):
                    pe.wait_ge(clsqP, gbase[i] + gi + 1)
                    for c in range(10 + clsd, 30, 2):
                        mm(c, 8, gi)
                # wave A: coor (sqC+rm01) then conf+noobj (sqA+rm_done)
                pe.wait_ge(sqC, i + 1)
                pe.wait_ge(rm01, i + 1)
                for gi in range(g):
                    mm(0, 0, gi)
                    mm(2, 2, gi)
                    mm(4, 0, gi)
                    mm(6, 2, gi)
                pe.wait_ge(sqA, i + 1)
                pe.wait_ge(rm_done, i + 1)
                for gi in range(g):
                    mm(8, 4, gi)
                    lastmm = (i == nchunk - 1 and gi == g - 1)
                    r = mm(30, 6, gi, last=lastmm)
                    if gi == g - 1:
                        r.then_inc(pe_done, 1)

    return nc


_NC_CACHE = {}


def _get_nc():
    if "nc" not in _NC_CACHE:
        _NC_CACHE["nc"] = build_nc()
    return _NC_CACHE["nc"]


def _to_bf16_repack(pred, labels):
    import ml_dtypes

    bf = ml_dtypes.bfloat16
    p = np.ascontiguousarray(pred, dtype=np.float32).reshape(B_TOTAL, 30, K)
    l = np.ascontiguousarray(labels, dtype=np.float32).reshape(B_TOTAL, 30, K)
    pb = p.astype(bf)
    lb = l.astype(bf)
    # obj channel: keep the ==1.0 test exact under rounding
    l4 = l[:, 4, :]
    lb4 = lb[:, 4, :]
    bad = (l4 != np.float32(1.0)) & (lb4.astype(np.float32) == np.float32(1.0))
    if bad.any():
        lb4[bad] = bf(0.99609375)
        lb[:, 4, :] = lb4
    xall = np.concatenate(
        [pb[:, PP_IDX[:PH], :], lb[:, LL_IDX[:LH], :],
         pb[:, PP_IDX[PH:], :], lb[:, LL_IDX[LH:], :]], axis=1)
    return np.ascontiguousarray(xall).reshape(B_TOTAL, ROWX)


def run_device(pred, labels, trace=False):
    from concourse.bass_utils import run_bass_kernel_spmd

    nc = _get_nc()
    xrp = _to_bf16_repack(pred, labels)
    in_maps = []
    for c in range(NCORES):
        rows = slice(c * B_CORE, (c + 1) * B_CORE)
        in_maps.append({"x": xrp[rows]})
    res = run_bass_kernel_spmd(nc, in_maps, list(range(NCORES)), trace=trace)
    total = 0.0
    for c in range(NCORES):
        m = res.results[c]["out2"].astype(np.float64)
        total += float(np.trace(m[0:98, 0:98]))
    loss = np.float32(total / B_TOTAL)
    return loss, res


def kernel(pred, labels):
    loss, _ = run_device(pred, labels, trace=False)
    return np.array(loss, dtype=np.float32)


if __name__ == "__main__":
    rng = np.random.default_rng(0)
    p = rng.random((B_TOTAL, 30, 7, 7), dtype=np.float32)
    l = rng.random((B_TOTAL, 30, 7, 7), dtype=np.float32)
    l[:, 4] = (rng.random((B_TOTAL, 7, 7)) < 0.3).astype(np.float32)
    print(kernel(p, l))
